# revision 29
# baseline (speedup 1.0000x reference)
"""Trainium2 Bass kernel for a causal attention head block (B=4, T=2048, C=2048,
H=16, D=128) with RoPE (single fixed position, folded into weights on host).

Sharding: 8 cores = 4 batches x 2 head-groups (8 heads each).
Per core: QKV projection, causal attention with exp-softmax (no max
subtraction -- scores are small), out-projection partial. Host sums the two
per-batch partials and adds the folded bias.

Data types: matmul operands in bf16/f16 (same PE speed as f32r, half the
DMA/SBUF), accumulation in f32 PSUM. Softmax denominator is accumulated on
the DVE in f16 (2x mode) so the PE runs one ones-matmul per 512-query chunk
instead of one per exp tile.
"""
import numpy as np

B, T, C, H, D = 4, 2048, 2048, 16, 128
ROPE_BASE = 10000.0
HG = H // 2            # heads per core: 8
JQ = HG * D            # 1024 q (or k, or v) channels per core
NCORES = 8
NCT = C // 128         # 16 contraction tiles
NTT = T // 128         # 16 token tiles
NTC = T // 512         # 4 token chunks of 512

_CACHE = {}


def _build_nc():
    import concourse.bass as bass
    import concourse.mybir as mybir
    import concourse.tile as tile
    from concourse import bacc

    f32, f32r = mybir.dt.float32, mybir.dt.float32r
    f16, bf16 = mybir.dt.float16, mybir.dt.bfloat16
    ds, ts = bass.ds, bass.ts
    Exp = mybir.ActivationFunctionType.Exp
    Ident = mybir.ActivationFunctionType.Identity
    mult = mybir.AluOpType.mult
    add = mybir.AluOpType.add

    nc = bacc.Bacc("TRN2", target_bir_lowering=False, debug=False)
    xT = nc.dram_tensor("xT", [C, T], bf16, kind="ExternalInput").ap()
    WqkT = nc.dram_tensor("WqkT", [C, 2 * JQ], bf16, kind="ExternalInput").ap()
    WvT = nc.dram_tensor("WvT", [C, JQ], bf16, kind="ExternalInput").ap()
    WoT = nc.dram_tensor("WoT", [JQ, C], bf16, kind="ExternalInput").ap()
    bq = nc.dram_tensor("bq", [JQ, 1], f32, kind="ExternalInput").ap()
    msk = nc.dram_tensor("msk", [128, 4 * 512], f16, kind="ExternalInput").ap()
    ones16_in = nc.dram_tensor("ones16_in", [128, 2], f16, kind="ExternalInput").ap()
    ones_in = nc.dram_tensor("ones_in", [128, 128], f32r, kind="ExternalInput").ap()
    qk_sp = nc.dram_tensor("qk_sp", [2 * JQ, T], bf16).ap()  # [q;k]^T spill
    v_sp = nc.dram_tensor("v_sp", [T, JQ], f16).ap()         # V spill [t, jv]
    o = nc.dram_tensor("o", [T, C], bf16, kind="ExternalOutput").ap()

    with tile.TileContext(nc) as tc:
        with tc.tile_pool(name="const", bufs=1) as cpool:
            mask_t = cpool.tile([128, 4 * 512], f16, tag="mask")
            nc.sync.dma_start(mask_t[:], msk[:])
            ones2d = cpool.tile([128, 128], f32r, tag="ones2d")
            nc.sync.dma_start(ones2d[:], ones_in[:])
            ones16 = cpool.tile([128, 2], f16, tag="ones16")
            nc.sync.dma_start(ones16[:], ones16_in[:])
            ones_col16 = ones16[:, 0:1]
            ones_row16 = mask_t[0:1, 384:512]
            ones_row = ones2d[0:1, :]
            bq_t = []
            for j in range(JQ // 128):
                t_ = cpool.tile([128, 1], f32, tag=f"bq{j}")
                nc.sync.dma_start(t_[:], bq[ts(j, 128), :])
                bq_t.append(t_)

            # ---------------- Phase A: projections ----------------
            # Head-0 attention inputs live below the xt stack slot so their
            # DMAs can prefetch during phase A (disjoint SBUF space).
            kv0cm = tc.tile_pool(name="kv0", bufs=1)
            kv0pool = kv0cm.__enter__()
            kt0 = kv0pool.tile([128, T], bf16, tag="kt0", name="kt0")
            vh0 = kv0pool.tile([128, T], f16, tag="vh0", name="vh0")
            qc0 = kv0pool.tile([128, 512], bf16, tag="qc0", name="qc0")
            with tc.tile_pool(name="xt", bufs=1) as xpool:
                # A-V first: V[t, jv] = x^T[c, t].T @ Wv^T[c, jv].
                # Interleave xT and first-chunk WvT DMAs so PE starts early.
                xt = [None] * NCT
                with tc.tile_pool(name="wv", bufs=19) as wvpool, \
                     tc.tile_pool(name="vst", bufs=4) as vspool, \
                     tc.tile_pool(name="psV", bufs=8, space="PSUM") as psvpool:
                    for vch in range(JQ // 512):     # 2 chunks of 512
                        wvs = []
                        for ci in range(NCT):
                            if vch == 0:
                                t_ = xpool.tile([128, T], bf16, tag=f"x{ci}",
                                                name=f"x{ci}")
                                # first token-column block right away so the
                                # tt=0 accumulation chain unblocks after
                                # ~3 MB of DMA (x[:,0:128] + wv only)
                                nc.sync.dma_start(t_[:, 0:128],
                                                 xT[ts(ci, 128), 0:128])
                                xt[ci] = t_
                            w_ = wvpool.tile([128, 512], bf16, tag="wv",
                                             name=f"wv{vch}_{ci}")
                            nc.sync.dma_start(
                                w_[:], WvT[ts(ci, 128), ds(vch * 512, 512)])
                            wvs.append(w_)
                        if vch == 0:
                            for ci in range(NCT):
                                nc.sync.dma_start(xt[ci][:, 128:512],
                                                  xT[ts(ci, 128), 128:512])
                            for tcol in range(1, NTC):
                                for ci in range(NCT):
                                    nc.sync.dma_start(
                                        xt[ci][:, ts(tcol, 512)],
                                        xT[ts(ci, 128), ts(tcol, 512)])
                        for tt in range(NTT):
                            ps = psvpool.tile([128, 512], f32, tag="psv")
                            for ci in range(NCT):
                                nc.tensor.matmul(
                                    ps[:], xt[ci][:, ts(tt, 128)], wvs[ci][:],
                                    start=(ci == 0), stop=(ci == NCT - 1))
                            st = vspool.tile([128, 512], f16, tag="vst")
                            nc.vector.tensor_copy(st[:], ps[:])
                            nc.gpsimd.dma_start(
                                v_sp[ts(tt, 128), ds(vch * 512, 512)], st[:])
                # A-QK: qk^T[j, t] = Wqk^T[c, j].T @ x^T[c, t]  (+ bias on q)
                # Group order q0, k0, q1, k1 so attention heads 0-3 unblock
                # after two groups.
                with tc.tile_pool(name="wqk", bufs=18) as wpool, \
                     tc.tile_pool(name="qkst", bufs=6) as spool, \
                     tc.tile_pool(name="psA", bufs=8, space="PSUM") as pspool:
                    for jg_i, jg in enumerate((0, 2, 1, 3)):
                        wts = []
                        for ci in range(NCT):
                            w_ = wpool.tile([128, 512], bf16, tag="w",
                                            name=f"w{jg}_{ci}")
                            nc.sync.dma_start(
                                w_[:], WqkT[ts(ci, 128), ds(jg * 512, 512)])
                            wts.append(w_)
                        for jj in range(4):
                            jt = jg * 4 + jj
                            pss_l = [pspool.tile([128, 512], f32, tag="ps",
                                                 name=f"ps{jt}_{t2}")
                                     for t2 in range(NTC)]
                            for ci in range(NCT):
                                for tch in range(NTC):
                                    nc.tensor.matmul(
                                        pss_l[tch][:], wts[ci][:, ts(jj, 128)],
                                        xt[ci][:, ts(tch, 512)],
                                        start=(ci == 0), stop=(ci == NCT - 1))
                            for tch in range(NTC):
                                st = spool.tile([128, 512], bf16, tag="st")
                                if jt < JQ // 128:   # q tile: bias add
                                    nc.scalar.activation(
                                        st[:], pss_l[tch][:], Ident,
                                        bias=bq_t[jt][:, 0:1])
                                else:                # k tile: plain copy
                                    nc.scalar.copy(st[:], pss_l[tch][:])
                                nc.gpsimd.dma_start(
                                    qk_sp[ts(jt, 128), ts(tch, 512)], st[:])
                        if jg_i == 1:
                            # prefetch head-0 attention inputs mid-phase-A
                            nc.sync.dma_start(
                                kt0[:], qk_sp[ds(JQ, 128), :])
                            nc.sync.dma_start(
                                vh0[:].rearrange("p (n d) -> p n d", d=128),
                                v_sp[:, ds(0, 128)].rearrange(
                                    "(n p) d -> p n d", p=128))
                            nc.sync.dma_start(qc0[:], qk_sp[ds(0, 128),
                                                            ds(0, 512)])

            # ---------------- Phase B: attention ----------------
            with tc.tile_pool(name="ysb", bufs=1) as ypool, \
                 tc.tile_pool(name="woc", bufs=1) as wopool:
                y_t = [ypool.tile([128, T], bf16, tag=f"y{h}", name=f"y{h}")
                       for h in range(HG)]
                # out-projection weights stream in during phase B so phase C
                # starts without a DMA stall
                wo_t = []
                for ch in range(HG):
                    w_ = wopool.tile([128, C], bf16, tag=f"wo{ch}",
                                     name=f"wo{ch}")
                    nc.sync.dma_start(w_[:], WoT[ts(ch, 128), :])
                    wo_t.append(w_)
                with tc.tile_pool(name="kv", bufs=4) as kvpool, \
                     tc.tile_pool(name="qc", bufs=3) as qcpool, \
                     tc.tile_pool(name="es", bufs=10) as espool, \
                     tc.tile_pool(name="acc", bufs=2) as accpool, \
                     tc.tile_pool(name="nrm", bufs=6) as npool, \
                     tc.tile_pool(name="psS", bufs=4, space="PSUM") as pss, \
                     tc.tile_pool(name="psY", bufs=2, space="PSUM") as psy, \
                     tc.tile_pool(name="psD", bufs=1, space="PSUM") as psd, \
                     tc.tile_pool(name="psB", bufs=1, space="PSUM") as psb:

                    # Slot-based software pipeline: one slot per score tile.
                    # The AV matmul for a tile runs LAG slots after its
                    # scores matmul so the exp (ACT) + mask (DVE) chain never
                    # stalls the PE. The softmax denominator is accumulated
                    # on the DVE (f16 2x) and reduced with one PE matmul per
                    # chunk; the normalization chain (dens -> reciprocal ->
                    # broadcast -> multiply) is spread over the next chunk's
                    # slots.
                    LAG = 3
                    avq = []        # (due_slot, es, ps_y, vh_sl, fi, la)
                    events = {}     # slot -> [thunk]

                    def at_slot(s, fn):
                        events.setdefault(s, []).append(fn)

                    def mk_dens(acc_, st):
                        def fn():
                            ps_d = psd.tile([1, 512], f32, tag="pd",
                                            name="pd")
                            nc.tensor.matmul(ps_d[:], ones_col16, acc_[:],
                                             start=True, stop=True)
                            st["ps_d"] = ps_d
                        return fn

                    def mk_rec(st):
                        def fn():
                            rec = npool.tile([1, 512], f32, tag="rec",
                                             name="rec")
                            nc.vector.reciprocal_approx_fast(
                                rec[:], st["ps_d"][:])
                            recr = npool.tile([1, 512], f16, tag="recr",
                                              name="recr")
                            nc.vector.tensor_copy(recr[:], rec[:])
                            st["recr"] = recr
                        return fn

                    def mk_bc(st):
                        def fn():
                            ps_b = psb.tile([128, 512], f32, tag="pb",
                                            name="pb")
                            nc.tensor.matmul(ps_b[:], ones_row16,
                                             st["recr"][:],
                                             start=True, stop=True)
                            bc = npool.tile([128, 512], f32r, tag="bc",
                                            name="bc")
                            nc.vector.tensor_copy(bc[:], ps_b[:])
                            st["bc"] = bc
                        return fn

                    def mk_mult(ps_y_, h_, ci_, st):
                        def fn():
                            nc.vector.tensor_tensor(
                                y_t[h_][:, ds(ci_ * 512, 512)], ps_y_[:],
                                st["bc"][:], mult)
                        return fn

                    def pump(slot):
                        for fn in events.pop(slot, []):
                            fn()
                        while avq and avq[0][0] <= slot:
                            _, e_, py_, vs_, fi_, la_ = avq.pop(0)
                            nc.tensor.matmul(py_[:], vs_, e_[:],
                                             start=fi_, stop=la_)

                    # prefetched attention inputs, one head / one chunk ahead
                    kts = {0: kt0}
                    vhs = {0: vh0}
                    qcs = {(0, 0): qc0}

                    def fetch_head(h):
                        if h >= HG or h in kts:
                            return
                        kt_ = kvpool.tile([128, T], bf16, tag="kt")
                        nc.sync.dma_start(
                            kt_[:], qk_sp[ds(JQ + h * 128, 128), :])
                        vh_ = kvpool.tile([128, T], f16, tag="vh")
                        nc.sync.dma_start(
                            vh_[:].rearrange("p (n d) -> p n d", d=128),
                            v_sp[:, ds(h * 128, 128)].rearrange(
                                "(n p) d -> p n d", p=128))
                        kts[h] = kt_
                        vhs[h] = vh_

                    def fetch_qc(h, ci):
                        if h >= HG or (h, ci) in qcs:
                            return
                        qc_ = qcpool.tile([128, 512], bf16, tag="qc")
                        nc.sync.dma_start(
                            qc_[:], qk_sp[ds(h * 128, 128), ds(ci * 512, 512)])
                        qcs[(h, ci)] = qc_

                    slot = 0
                    for h in range(HG):
                        fetch_head(h)
                        kt, vh = kts[h], vhs[h]
                        fetch_head(h + 1)
                        for ci in range(NTC):
                            fetch_qc(h, ci)
                            qc = qcs.pop((h, ci))
                            if ci + 1 < NTC:
                                fetch_qc(h, ci + 1)
                            else:
                                fetch_qc(h + 1, 0)
                            ps_y = psy.tile([128, 512], f32, tag="py")
                            acc = accpool.tile([128, 512], f16, tag="acc",
                                               name="acc")
                            njt = 4 * (ci + 1)
                            for jt in range(njt):
                                ps_s = pss.tile([128, 512], f32, tag="pss")
                                nc.tensor.matmul(ps_s[:], kt[:, ts(jt, 128)],
                                                 qc[:], start=True, stop=True)
                                pump(slot)
                                es = espool.tile([128, 512], f16, tag="es")
                                off = jt * 128 - ci * 512
                                if off > 0:
                                    # diagonal band: columns < off are fully
                                    # masked -- zero them and exp only the
                                    # valid range (saves ACT, the B-phase
                                    # bottleneck)
                                    nc.vector.memset(es[:, 0:off], 0.0)
                                    nc.scalar.activation(es[:, ds(off, 512 - off)],
                                                         ps_s[:, ds(off, 512 - off)],
                                                         Exp)
                                else:
                                    nc.scalar.activation(es[:], ps_s[:], Exp)
                                if off >= 0:
                                    # triangular 128-wide block at the
                                    # diagonal is the only mixed region
                                    oi = off // 128
                                    nc.vector.tensor_tensor(
                                        es[:, ds(off, 128)],
                                        es[:, ds(off, 128)],
                                        mask_t[:, ds(oi * 512 + off, 128)],
                                        mult)
                                if jt == 0:
                                    nc.vector.tensor_copy(acc[:], es[:])
                                elif off > 0:
                                    nc.vector.tensor_tensor(
                                        acc[:, ds(off, 512 - off)],
                                        acc[:, ds(off, 512 - off)],
                                        es[:, ds(off, 512 - off)], add)
                                else:
                                    nc.vector.tensor_tensor(acc[:], acc[:],
                                                            es[:], add)
                                avq.append((slot + LAG, es, ps_y,
                                            vh[:, ts(jt, 128)],
                                            jt == 0, jt == njt - 1))
                                slot += 1
                            st = {}
                            at_slot(slot + 1, mk_dens(acc, st))
                            at_slot(slot + 3, mk_rec(st))
                            at_slot(slot + 5, mk_bc(st))
                            at_slot(slot + 7, mk_mult(ps_y, h, ci, st))
                    # drain the pipeline
                    while avq or events:
                        pump(slot)
                        slot += 1

                # ---------------- Phase C: out-projection ----------------
                with tc.tile_pool(name="ost", bufs=6) as ospool, \
                     tc.tile_pool(name="psO", bufs=8, space="PSUM") as pso:
                    for tt in range(NTT):
                        po_l = [pso.tile([128, 512], f32, tag="po",
                                         name=f"po{tt}_{c2}")
                                for c2 in range(C // 512)]
                        for ch in range(HG):
                            for cch in range(C // 512):
                                nc.tensor.matmul(
                                    po_l[cch][:], y_t[ch][:, ts(tt, 128)],
                                    wo_t[ch][:, ds(cch * 512, 512)],
                                    start=(ch == 0), stop=(ch == HG - 1))
                        for cch in range(C // 512):
                            st = ospool.tile([128, 512], bf16, tag="ost")
                            nc.vector.tensor_copy(st[:], po_l[cch][:])
                            nc.gpsimd.dma_start(
                                o[ts(tt, 128), ds(cch * 512, 512)], st[:])
            kv0cm.__exit__(None, None, None)
    nc.compile()
    return nc


def _rope_matrix():
    inv_freq = 1.0 / (ROPE_BASE ** (np.arange(0, D, 2, dtype=np.float64) / D))
    freqs = np.float64(T) * inv_freq
    emb = np.concatenate([freqs, freqs])
    cos, sin = np.cos(emb), np.sin(emb)
    R = np.zeros((D, D))
    for j in range(D):
        R[j, j] = cos[j]
    for j in range(64):
        R[j, 2 * j + 1] += -sin[j]
    for j in range(64, 128):
        R[j, 2 * (j - 64)] += sin[j]
    return R


def _host_mask():
    m = np.zeros((128, 4 * 512), dtype=np.float16)
    ii = np.arange(512)
    jj = np.arange(128)
    for oi, off in enumerate((0, 128, 256, 384)):
        m[:, oi * 512:(oi + 1) * 512] = (jj[:, None] + off <= ii[None, :])
    return m


def kernel(x, Wqkv, bqkv, Wout, bout):
    import ml_dtypes
    from concourse.bass_utils import run_bass_kernel_spmd

    bfloat16 = ml_dtypes.bfloat16

    if "nc" not in _CACHE:
        _CACHE["nc"] = _build_nc()
    nc = _CACHE["nc"]

    x = np.asarray(x, dtype=np.float32)
    Wqkv64 = np.asarray(Wqkv, dtype=np.float64)
    bqkv64 = np.asarray(bqkv, dtype=np.float64)
    Wout64 = np.asarray(Wout, dtype=np.float64)
    bout64 = np.asarray(bout, dtype=np.float64)

    R = _rope_matrix()
    scale = 1.0 / np.sqrt(np.float64(D))
    Wq = Wqkv64[:C].reshape(H, D, C)
    Wk = Wqkv64[C:2 * C].reshape(H, D, C)
    Wv = Wqkv64[2 * C:].reshape(H, D, C)
    bqv = bqkv64[:C].reshape(H, D)
    bv = bqkv64[2 * C:]

    Wq_f = np.einsum('jk,hkc->hjc', R, Wq) * scale
    bq_f = np.einsum('jk,hk->hj', R, bqv) * scale
    Wk_f = np.einsum('jk,hkc->hjc', R, Wk)
    bias_final = (bout64 + Wout64 @ bv).astype(np.float32)

    mask = _host_mask()
    in_maps = []
    xTb = [np.ascontiguousarray(x[b].T).astype(bfloat16) for b in range(B)]
    shard = {}
    for hg in range(2):
        hs = slice(hg * HG, (hg + 1) * HG)
        wqk = np.concatenate(
            [Wq_f[hs].reshape(JQ, C), Wk_f[hs].reshape(JQ, C)], axis=0)
        shard[hg] = dict(
            WqkT=np.ascontiguousarray(wqk.T).astype(bfloat16),
            WvT=np.ascontiguousarray(
                Wv[hs].reshape(JQ, C).T).astype(bfloat16),
            WoT=np.ascontiguousarray(
                Wout64[:, hg * JQ:(hg + 1) * JQ].T).astype(bfloat16),
            bq=bq_f[hs].reshape(JQ, 1).astype(np.float32),
        )
    for core in range(NCORES):
        b, hg = core // 2, core % 2
        in_maps.append(dict(xT=xTb[b], msk=mask,
                            ones_in=np.ones((128, 128), dtype=np.float32),
                            ones16_in=np.ones((128, 2), dtype=np.float16),
                            **shard[hg]))

    res = run_bass_kernel_spmd(nc, in_maps, core_ids=list(range(NCORES)),
                               **_CACHE.get("run_kwargs", {}))
    _CACHE["last_result"] = res
    out = np.empty((B, T, C), dtype=np.float32)
    for b in range(B):
        out[b] = (res.results[2 * b]["o"].astype(np.float32)
                  + res.results[2 * b + 1]["o"].astype(np.float32)
                  + bias_final)
    return out


# revision 30
# speedup vs baseline: 1.0287x; 1.0287x over previous
"""Trainium2 Bass kernel for a causal attention head block (B=4, T=2048, C=2048,
H=16, D=128) with RoPE (single fixed position, folded into weights on host).

Sharding: 8 cores = 4 batches x 2 head-groups (8 heads each).
Per core: QKV projection, causal attention with exp-softmax (no max
subtraction -- scores are small), out-projection partial. Host sums the two
per-batch partials and adds the folded bias.

Data types: matmul operands in bf16/f16 (same PE speed as f32r, half the
DMA/SBUF), accumulation in f32 PSUM. Softmax denominator is accumulated on
the DVE in f16 (2x mode) so the PE runs one ones-matmul per 512-query chunk
instead of one per exp tile.
"""
import numpy as np

B, T, C, H, D = 4, 2048, 2048, 16, 128
ROPE_BASE = 10000.0
HG = H // 2            # heads per core: 8
JQ = HG * D            # 1024 q (or k, or v) channels per core
NCORES = 8
NCT = C // 128         # 16 contraction tiles
NTT = T // 128         # 16 token tiles
NTC = T // 512         # 4 token chunks of 512

_CACHE = {}


def _build_nc():
    import concourse.bass as bass
    import concourse.mybir as mybir
    import concourse.tile as tile
    from concourse import bacc

    f32, f32r = mybir.dt.float32, mybir.dt.float32r
    f16, bf16 = mybir.dt.float16, mybir.dt.bfloat16
    ds, ts = bass.ds, bass.ts
    Exp = mybir.ActivationFunctionType.Exp
    Ident = mybir.ActivationFunctionType.Identity
    mult = mybir.AluOpType.mult
    add = mybir.AluOpType.add

    nc = bacc.Bacc("TRN2", target_bir_lowering=False, debug=False)
    xT = nc.dram_tensor("xT", [C, T], bf16, kind="ExternalInput").ap()
    WqkT = nc.dram_tensor("WqkT", [C, 2 * JQ], bf16, kind="ExternalInput").ap()
    WvT = nc.dram_tensor("WvT", [C, JQ], bf16, kind="ExternalInput").ap()
    WoT = nc.dram_tensor("WoT", [JQ, C], bf16, kind="ExternalInput").ap()
    bq = nc.dram_tensor("bq", [JQ, 1], f32, kind="ExternalInput").ap()
    msk = nc.dram_tensor("msk", [128, 4 * 512], f16, kind="ExternalInput").ap()
    ones16_in = nc.dram_tensor("ones16_in", [128, 2], f16, kind="ExternalInput").ap()
    ones_in = nc.dram_tensor("ones_in", [128, 128], f32r, kind="ExternalInput").ap()
    qk_sp = nc.dram_tensor("qk_sp", [2 * JQ, T], bf16).ap()  # [q;k]^T spill
    v_sp = nc.dram_tensor("v_sp", [T, JQ], f16).ap()         # V spill [t, jv]
    o = nc.dram_tensor("o", [T, C], bf16, kind="ExternalOutput").ap()

    with tile.TileContext(nc) as tc:
        with tc.tile_pool(name="const", bufs=1) as cpool:
            mask_t = cpool.tile([128, 4 * 512], f16, tag="mask")
            nc.sync.dma_start(mask_t[:], msk[:])
            ones2d = cpool.tile([128, 128], f32r, tag="ones2d")
            nc.sync.dma_start(ones2d[:], ones_in[:])
            ones16 = cpool.tile([128, 2], f16, tag="ones16")
            nc.sync.dma_start(ones16[:], ones16_in[:])
            ones_col16 = ones16[:, 0:1]
            ones_row16 = mask_t[0:1, 384:512]
            ones_row = ones2d[0:1, :]
            bq_t = []
            for j in range(JQ // 128):
                t_ = cpool.tile([128, 1], f32, tag=f"bq{j}")
                nc.sync.dma_start(t_[:], bq[ts(j, 128), :])
                bq_t.append(t_)

            # ---------------- Phase A: projections ----------------
            # Head-0 attention inputs live below the xt stack slot so their
            # DMAs can prefetch during phase A (disjoint SBUF space).
            kv0cm = tc.tile_pool(name="kv0", bufs=1)
            kv0pool = kv0cm.__enter__()
            kt0 = kv0pool.tile([128, T], bf16, tag="kt0", name="kt0")
            vh0 = kv0pool.tile([128, T], f16, tag="vh0", name="vh0")
            qc0 = kv0pool.tile([128, 512], bf16, tag="qc0", name="qc0")
            with tc.tile_pool(name="xt", bufs=1) as xpool:
                # A-V first: V[t, jv] = x^T[c, t].T @ Wv^T[c, jv].
                # Interleave xT and first-chunk WvT DMAs so PE starts early.
                xt = [None] * NCT
                with tc.tile_pool(name="wv", bufs=19) as wvpool, \
                     tc.tile_pool(name="vst", bufs=4) as vspool, \
                     tc.tile_pool(name="psV", bufs=8, space="PSUM") as psvpool:
                    for vch in range(JQ // 512):     # 2 chunks of 512
                        wvs = []
                        for ci in range(NCT):
                            if vch == 0:
                                t_ = xpool.tile([128, T], bf16, tag=f"x{ci}",
                                                name=f"x{ci}")
                                # first token-column block right away so the
                                # tt=0 accumulation chain unblocks after
                                # ~3 MB of DMA (x[:,0:128] + wv only)
                                nc.sync.dma_start(t_[:, 0:128],
                                                 xT[ts(ci, 128), 0:128])
                                xt[ci] = t_
                            w_ = wvpool.tile([128, 512], bf16, tag="wv",
                                             name=f"wv{vch}_{ci}")
                            nc.sync.dma_start(
                                w_[:], WvT[ts(ci, 128), ds(vch * 512, 512)])
                            wvs.append(w_)
                        if vch == 0:
                            for ci in range(NCT):
                                nc.sync.dma_start(xt[ci][:, 128:512],
                                                  xT[ts(ci, 128), 128:512])
                            for tcol in range(1, NTC):
                                for ci in range(NCT):
                                    nc.sync.dma_start(
                                        xt[ci][:, ts(tcol, 512)],
                                        xT[ts(ci, 128), ts(tcol, 512)])
                        for tt in range(NTT):
                            ps = psvpool.tile([128, 512], f32, tag="psv")
                            for ci in range(NCT):
                                nc.tensor.matmul(
                                    ps[:], xt[ci][:, ts(tt, 128)], wvs[ci][:],
                                    start=(ci == 0), stop=(ci == NCT - 1))
                            st = vspool.tile([128, 512], f16, tag="vst")
                            nc.vector.tensor_copy(st[:], ps[:])
                            nc.gpsimd.dma_start(
                                v_sp[ts(tt, 128), ds(vch * 512, 512)], st[:])
                # A-QK: qk^T[j, t] = Wqk^T[c, j].T @ x^T[c, t]  (+ bias on q)
                # Group order q0, k0, q1, k1 so attention heads 0-3 unblock
                # after two groups.
                with tc.tile_pool(name="wqk", bufs=18) as wpool, \
                     tc.tile_pool(name="qkst", bufs=6) as spool, \
                     tc.tile_pool(name="psA", bufs=8, space="PSUM") as pspool:
                    for jg_i, jg in enumerate((0, 2, 1, 3)):
                        wts = []
                        for ci in range(NCT):
                            w_ = wpool.tile([128, 512], bf16, tag="w",
                                            name=f"w{jg}_{ci}")
                            nc.sync.dma_start(
                                w_[:], WqkT[ts(ci, 128), ds(jg * 512, 512)])
                            wts.append(w_)
                        for jj in range(4):
                            jt = jg * 4 + jj
                            pss_l = [pspool.tile([128, 512], f32, tag="ps",
                                                 name=f"ps{jt}_{t2}")
                                     for t2 in range(NTC)]
                            for ci in range(NCT):
                                for tch in range(NTC):
                                    nc.tensor.matmul(
                                        pss_l[tch][:], wts[ci][:, ts(jj, 128)],
                                        xt[ci][:, ts(tch, 512)],
                                        start=(ci == 0), stop=(ci == NCT - 1))
                            for tch in range(NTC):
                                st = spool.tile([128, 512], bf16, tag="st")
                                if jt < JQ // 128:   # q tile: bias add
                                    nc.scalar.activation(
                                        st[:], pss_l[tch][:], Ident,
                                        bias=bq_t[jt][:, 0:1])
                                else:                # k tile: plain copy
                                    nc.scalar.copy(st[:], pss_l[tch][:])
                                nc.gpsimd.dma_start(
                                    qk_sp[ts(jt, 128), ts(tch, 512)], st[:])
                        if jg_i == 1:
                            # prefetch head-0 attention inputs mid-phase-A
                            nc.sync.dma_start(
                                kt0[:], qk_sp[ds(JQ, 128), :])
                            nc.sync.dma_start(
                                vh0[:].rearrange("p (n d) -> p n d", d=128),
                                v_sp[:, ds(0, 128)].rearrange(
                                    "(n p) d -> p n d", p=128))
                            nc.sync.dma_start(qc0[:], qk_sp[ds(0, 128),
                                                            ds(0, 512)])

            # ---------------- Phase B: attention ----------------
            with tc.tile_pool(name="ysb", bufs=1) as ypool, \
                 tc.tile_pool(name="woc", bufs=1) as wopool:
                y_t = [ypool.tile([128, T], bf16, tag=f"y{h}", name=f"y{h}")
                       for h in range(HG)]
                # out-projection weights stream in during phase B so phase C
                # starts without a DMA stall
                wo_t = []
                for ch in range(HG):
                    w_ = wopool.tile([128, C], bf16, tag=f"wo{ch}",
                                     name=f"wo{ch}")
                    nc.sync.dma_start(w_[:], WoT[ts(ch, 128), :])
                    wo_t.append(w_)
                with tc.tile_pool(name="kv", bufs=4) as kvpool, \
                     tc.tile_pool(name="qc", bufs=3) as qcpool, \
                     tc.tile_pool(name="es", bufs=10) as espool, \
                     tc.tile_pool(name="acc", bufs=2) as accpool, \
                     tc.tile_pool(name="nrm", bufs=6) as npool, \
                     tc.tile_pool(name="psS", bufs=4, space="PSUM") as pss, \
                     tc.tile_pool(name="psY", bufs=2, space="PSUM") as psy, \
                     tc.tile_pool(name="psD", bufs=1, space="PSUM") as psd, \
                     tc.tile_pool(name="psB", bufs=1, space="PSUM") as psb:

                    # Slot-based software pipeline: one slot per score tile.
                    # The AV matmul for a tile runs LAG slots after its
                    # scores matmul so the exp (ACT) + mask (DVE) chain never
                    # stalls the PE. The softmax denominator is accumulated
                    # on the DVE (f16 2x) and reduced with one PE matmul per
                    # chunk; the normalization chain (dens -> reciprocal ->
                    # broadcast -> multiply) is spread over the next chunk's
                    # slots.
                    LAG = 3
                    avq = []        # (due_slot, es, ps_y, vh_sl, fi, la)
                    events = {}     # slot -> [thunk]

                    def at_slot(s, fn):
                        events.setdefault(s, []).append(fn)

                    def mk_dens(acc_, st):
                        def fn():
                            ps_d = psd.tile([1, 512], f32, tag="pd",
                                            name="pd")
                            nc.tensor.matmul(ps_d[:], ones_col16, acc_[:],
                                             start=True, stop=True)
                            st["ps_d"] = ps_d
                        return fn

                    def mk_rec(st):
                        def fn():
                            rec = npool.tile([1, 512], f32, tag="rec",
                                             name="rec")
                            nc.vector.reciprocal_approx_fast(
                                rec[:], st["ps_d"][:])
                            recr = npool.tile([1, 512], f16, tag="recr",
                                              name="recr")
                            nc.vector.tensor_copy(recr[:], rec[:])
                            st["recr"] = recr
                        return fn

                    def mk_bc(st):
                        def fn():
                            ps_b = psb.tile([128, 512], f32, tag="pb",
                                            name="pb")
                            nc.tensor.matmul(ps_b[:], ones_row16,
                                             st["recr"][:],
                                             start=True, stop=True)
                            bc = npool.tile([128, 512], f32r, tag="bc",
                                            name="bc")
                            nc.scalar.copy(bc[:], ps_b[:])
                            st["bc"] = bc
                        return fn

                    def mk_mult(ps_y_, h_, ci_, st):
                        def fn():
                            nc.vector.tensor_tensor(
                                y_t[h_][:, ds(ci_ * 512, 512)], ps_y_[:],
                                st["bc"][:], mult)
                        return fn

                    def pump(slot):
                        for fn in events.pop(slot, []):
                            fn()
                        while avq and avq[0][0] <= slot:
                            _, e_, py_, vs_, fi_, la_ = avq.pop(0)
                            nc.tensor.matmul(py_[:], vs_, e_[:],
                                             start=fi_, stop=la_)

                    # prefetched attention inputs, one head / one chunk ahead
                    kts = {0: kt0}
                    vhs = {0: vh0}
                    qcs = {(0, 0): qc0}

                    def fetch_head(h):
                        if h >= HG or h in kts:
                            return
                        kt_ = kvpool.tile([128, T], bf16, tag="kt")
                        nc.sync.dma_start(
                            kt_[:], qk_sp[ds(JQ + h * 128, 128), :])
                        vh_ = kvpool.tile([128, T], f16, tag="vh")
                        nc.sync.dma_start(
                            vh_[:].rearrange("p (n d) -> p n d", d=128),
                            v_sp[:, ds(h * 128, 128)].rearrange(
                                "(n p) d -> p n d", p=128))
                        kts[h] = kt_
                        vhs[h] = vh_

                    def fetch_qc(h, ci):
                        if h >= HG or (h, ci) in qcs:
                            return
                        qc_ = qcpool.tile([128, 512], bf16, tag="qc")
                        nc.sync.dma_start(
                            qc_[:], qk_sp[ds(h * 128, 128), ds(ci * 512, 512)])
                        qcs[(h, ci)] = qc_

                    slot = 0
                    for h in range(HG):
                        fetch_head(h)
                        kt, vh = kts[h], vhs[h]
                        fetch_head(h + 1)
                        for ci in range(NTC):
                            fetch_qc(h, ci)
                            qc = qcs.pop((h, ci))
                            if ci + 1 < NTC:
                                fetch_qc(h, ci + 1)
                            else:
                                fetch_qc(h + 1, 0)
                            ps_y = psy.tile([128, 512], f32, tag="py")
                            acc = accpool.tile([128, 512], f16, tag="acc",
                                               name="acc")
                            njt = 4 * (ci + 1)
                            for jt in range(njt):
                                ps_s = pss.tile([128, 512], f32, tag="pss")
                                nc.tensor.matmul(ps_s[:], kt[:, ts(jt, 128)],
                                                 qc[:], start=True, stop=True)
                                pump(slot)
                                es = espool.tile([128, 512], f16, tag="es")
                                off = jt * 128 - ci * 512
                                if off > 0:
                                    # diagonal band: columns < off are fully
                                    # masked -- zero them and exp only the
                                    # valid range (saves ACT, the B-phase
                                    # bottleneck)
                                    nc.vector.memset(es[:, 0:off], 0.0)
                                    nc.scalar.activation(es[:, ds(off, 512 - off)],
                                                         ps_s[:, ds(off, 512 - off)],
                                                         Exp)
                                else:
                                    nc.scalar.activation(es[:], ps_s[:], Exp)
                                if off >= 0:
                                    # triangular 128-wide block at the
                                    # diagonal is the only mixed region
                                    oi = off // 128
                                    nc.vector.tensor_tensor(
                                        es[:, ds(off, 128)],
                                        es[:, ds(off, 128)],
                                        mask_t[:, ds(oi * 512 + off, 128)],
                                        mult)
                                if jt == 0:
                                    nc.vector.tensor_copy(acc[:], es[:])
                                elif off > 0:
                                    nc.vector.tensor_tensor(
                                        acc[:, ds(off, 512 - off)],
                                        acc[:, ds(off, 512 - off)],
                                        es[:, ds(off, 512 - off)], add)
                                else:
                                    nc.vector.tensor_tensor(acc[:], acc[:],
                                                            es[:], add)
                                avq.append((slot + LAG, es, ps_y,
                                            vh[:, ts(jt, 128)],
                                            jt == 0, jt == njt - 1))
                                slot += 1
                            st = {}
                            at_slot(slot + 1, mk_dens(acc, st))
                            at_slot(slot + 3, mk_rec(st))
                            at_slot(slot + 5, mk_bc(st))
                            at_slot(slot + 7, mk_mult(ps_y, h, ci, st))
                    # drain the pipeline
                    while avq or events:
                        pump(slot)
                        slot += 1

                # ---------------- Phase C: out-projection ----------------
                with tc.tile_pool(name="ost", bufs=6) as ospool, \
                     tc.tile_pool(name="psO", bufs=8, space="PSUM") as pso:
                    for tt in range(NTT):
                        po_l = [pso.tile([128, 512], f32, tag="po",
                                         name=f"po{tt}_{c2}")
                                for c2 in range(C // 512)]
                        for ch in range(HG):
                            for cch in range(C // 512):
                                nc.tensor.matmul(
                                    po_l[cch][:], y_t[ch][:, ts(tt, 128)],
                                    wo_t[ch][:, ds(cch * 512, 512)],
                                    start=(ch == 0), stop=(ch == HG - 1))
                        for cch in range(C // 512):
                            st = ospool.tile([128, 512], bf16, tag="ost")
                            nc.vector.tensor_copy(st[:], po_l[cch][:])
                            nc.gpsimd.dma_start(
                                o[ts(tt, 128), ds(cch * 512, 512)], st[:])
            kv0cm.__exit__(None, None, None)
    nc.compile()
    return nc


def _rope_matrix():
    inv_freq = 1.0 / (ROPE_BASE ** (np.arange(0, D, 2, dtype=np.float64) / D))
    freqs = np.float64(T) * inv_freq
    emb = np.concatenate([freqs, freqs])
    cos, sin = np.cos(emb), np.sin(emb)
    R = np.zeros((D, D))
    for j in range(D):
        R[j, j] = cos[j]
    for j in range(64):
        R[j, 2 * j + 1] += -sin[j]
    for j in range(64, 128):
        R[j, 2 * (j - 64)] += sin[j]
    return R


def _host_mask():
    m = np.zeros((128, 4 * 512), dtype=np.float16)
    ii = np.arange(512)
    jj = np.arange(128)
    for oi, off in enumerate((0, 128, 256, 384)):
        m[:, oi * 512:(oi + 1) * 512] = (jj[:, None] + off <= ii[None, :])
    return m


def kernel(x, Wqkv, bqkv, Wout, bout):
    import ml_dtypes
    from concourse.bass_utils import run_bass_kernel_spmd

    bfloat16 = ml_dtypes.bfloat16

    if "nc" not in _CACHE:
        _CACHE["nc"] = _build_nc()
    nc = _CACHE["nc"]

    x = np.asarray(x, dtype=np.float32)
    Wqkv64 = np.asarray(Wqkv, dtype=np.float64)
    bqkv64 = np.asarray(bqkv, dtype=np.float64)
    Wout64 = np.asarray(Wout, dtype=np.float64)
    bout64 = np.asarray(bout, dtype=np.float64)

    R = _rope_matrix()
    scale = 1.0 / np.sqrt(np.float64(D))
    Wq = Wqkv64[:C].reshape(H, D, C)
    Wk = Wqkv64[C:2 * C].reshape(H, D, C)
    Wv = Wqkv64[2 * C:].reshape(H, D, C)
    bqv = bqkv64[:C].reshape(H, D)
    bv = bqkv64[2 * C:]

    Wq_f = np.einsum('jk,hkc->hjc', R, Wq) * scale
    bq_f = np.einsum('jk,hk->hj', R, bqv) * scale
    Wk_f = np.einsum('jk,hkc->hjc', R, Wk)
    bias_final = (bout64 + Wout64 @ bv).astype(np.float32)

    mask = _host_mask()
    in_maps = []
    xTb = [np.ascontiguousarray(x[b].T).astype(bfloat16) for b in range(B)]
    shard = {}
    for hg in range(2):
        hs = slice(hg * HG, (hg + 1) * HG)
        wqk = np.concatenate(
            [Wq_f[hs].reshape(JQ, C), Wk_f[hs].reshape(JQ, C)], axis=0)
        shard[hg] = dict(
            WqkT=np.ascontiguousarray(wqk.T).astype(bfloat16),
            WvT=np.ascontiguousarray(
                Wv[hs].reshape(JQ, C).T).astype(bfloat16),
            WoT=np.ascontiguousarray(
                Wout64[:, hg * JQ:(hg + 1) * JQ].T).astype(bfloat16),
            bq=bq_f[hs].reshape(JQ, 1).astype(np.float32),
        )
    for core in range(NCORES):
        b, hg = core // 2, core % 2
        in_maps.append(dict(xT=xTb[b], msk=mask,
                            ones_in=np.ones((128, 128), dtype=np.float32),
                            ones16_in=np.ones((128, 2), dtype=np.float16),
                            **shard[hg]))

    res = run_bass_kernel_spmd(nc, in_maps, core_ids=list(range(NCORES)),
                               **_CACHE.get("run_kwargs", {}))
    _CACHE["last_result"] = res
    out = np.empty((B, T, C), dtype=np.float32)
    for b in range(B):
        out[b] = (res.results[2 * b]["o"].astype(np.float32)
                  + res.results[2 * b + 1]["o"].astype(np.float32)
                  + bias_final)
    return out


# revision 31
# speedup vs baseline: 1.0427x; 1.0136x over previous
"""Trainium2 Bass kernel for a causal attention head block (B=4, T=2048, C=2048,
H=16, D=128) with RoPE (single fixed position, folded into weights on host).

Sharding: 8 cores = 4 batches x 2 head-groups (8 heads each).
Per core: QKV projection, causal attention with exp-softmax (no max
subtraction -- scores are small), out-projection partial. Host sums the two
per-batch partials and adds the folded bias.

Data types: matmul operands in bf16/f16 (same PE speed as f32r, half the
DMA/SBUF), accumulation in f32 PSUM. Softmax denominator is accumulated on
the DVE in f16 (2x mode) so the PE runs one ones-matmul per 512-query chunk
instead of one per exp tile.
"""
import numpy as np

B, T, C, H, D = 4, 2048, 2048, 16, 128
ROPE_BASE = 10000.0
HG = H // 2            # heads per core: 8
JQ = HG * D            # 1024 q (or k, or v) channels per core
NCORES = 8
NCT = C // 128         # 16 contraction tiles
NTT = T // 128         # 16 token tiles
NTC = T // 512         # 4 token chunks of 512

_CACHE = {}


def _build_nc():
    import concourse.bass as bass
    import concourse.mybir as mybir
    import concourse.tile as tile
    from concourse import bacc

    f32, f32r = mybir.dt.float32, mybir.dt.float32r
    f16, bf16 = mybir.dt.float16, mybir.dt.bfloat16
    ds, ts = bass.ds, bass.ts
    Exp = mybir.ActivationFunctionType.Exp
    Ident = mybir.ActivationFunctionType.Identity
    mult = mybir.AluOpType.mult
    add = mybir.AluOpType.add

    nc = bacc.Bacc("TRN2", target_bir_lowering=False, debug=False)
    xT = nc.dram_tensor("xT", [C, T], bf16, kind="ExternalInput").ap()
    WqkT = nc.dram_tensor("WqkT", [C, 2 * JQ], bf16, kind="ExternalInput").ap()
    WvT = nc.dram_tensor("WvT", [C, JQ], bf16, kind="ExternalInput").ap()
    WoT = nc.dram_tensor("WoT", [JQ, C], bf16, kind="ExternalInput").ap()
    bq = nc.dram_tensor("bq", [JQ, 1], f32, kind="ExternalInput").ap()
    msk = nc.dram_tensor("msk", [128, 4 * 512], f16, kind="ExternalInput").ap()
    ones16_in = nc.dram_tensor("ones16_in", [128, 2], f16, kind="ExternalInput").ap()
    ones_in = nc.dram_tensor("ones_in", [128, 128], f32r, kind="ExternalInput").ap()
    qk_sp = nc.dram_tensor("qk_sp", [2 * JQ, T], bf16).ap()  # [q;k]^T spill
    v_sp = nc.dram_tensor("v_sp", [T, JQ], f16).ap()         # V spill [t, jv]
    o = nc.dram_tensor("o", [T, C], bf16, kind="ExternalOutput").ap()

    with tile.TileContext(nc) as tc:
        with tc.tile_pool(name="const", bufs=1) as cpool:
            mask_t = cpool.tile([128, 4 * 512], f16, tag="mask")
            nc.sync.dma_start(mask_t[:], msk[:])
            ones2d = cpool.tile([128, 128], f32r, tag="ones2d")
            nc.sync.dma_start(ones2d[:], ones_in[:])
            ones16 = cpool.tile([128, 2], f16, tag="ones16")
            nc.sync.dma_start(ones16[:], ones16_in[:])
            ones_sq16 = mask_t[:, 384:512]
            ones_row = ones2d[0:1, :]
            bq_t = []
            for j in range(JQ // 128):
                t_ = cpool.tile([128, 1], f32, tag=f"bq{j}")
                nc.sync.dma_start(t_[:], bq[ts(j, 128), :])
                bq_t.append(t_)

            # ---------------- Phase A: projections ----------------
            # Head-0 attention inputs live below the xt stack slot so their
            # DMAs can prefetch during phase A (disjoint SBUF space).
            kv0cm = tc.tile_pool(name="kv0", bufs=1)
            kv0pool = kv0cm.__enter__()
            kt0 = kv0pool.tile([128, T], bf16, tag="kt0", name="kt0")
            vh0 = kv0pool.tile([128, T], f16, tag="vh0", name="vh0")
            qc0 = kv0pool.tile([128, 512], bf16, tag="qc0", name="qc0")
            with tc.tile_pool(name="xt", bufs=1) as xpool:
                # A-V first: V[t, jv] = x^T[c, t].T @ Wv^T[c, jv].
                # Interleave xT and first-chunk WvT DMAs so PE starts early.
                xt = [None] * NCT
                with tc.tile_pool(name="wv", bufs=19) as wvpool, \
                     tc.tile_pool(name="vst", bufs=4) as vspool, \
                     tc.tile_pool(name="psV", bufs=8, space="PSUM") as psvpool:
                    for vch in range(JQ // 512):     # 2 chunks of 512
                        wvs = []
                        for ci in range(NCT):
                            if vch == 0:
                                t_ = xpool.tile([128, T], bf16, tag=f"x{ci}",
                                                name=f"x{ci}")
                                # first token-column block right away so the
                                # tt=0 accumulation chain unblocks after
                                # ~3 MB of DMA (x[:,0:128] + wv only)
                                nc.sync.dma_start(t_[:, 0:128],
                                                 xT[ts(ci, 128), 0:128])
                                xt[ci] = t_
                            w_ = wvpool.tile([128, 512], bf16, tag="wv",
                                             name=f"wv{vch}_{ci}")
                            nc.sync.dma_start(
                                w_[:], WvT[ts(ci, 128), ds(vch * 512, 512)])
                            wvs.append(w_)
                        if vch == 0:
                            for ci in range(NCT):
                                nc.sync.dma_start(xt[ci][:, 128:512],
                                                  xT[ts(ci, 128), 128:512])
                            for tcol in range(1, NTC):
                                for ci in range(NCT):
                                    nc.sync.dma_start(
                                        xt[ci][:, ts(tcol, 512)],
                                        xT[ts(ci, 128), ts(tcol, 512)])
                        for tt in range(NTT):
                            ps = psvpool.tile([128, 512], f32, tag="psv")
                            for ci in range(NCT):
                                nc.tensor.matmul(
                                    ps[:], xt[ci][:, ts(tt, 128)], wvs[ci][:],
                                    start=(ci == 0), stop=(ci == NCT - 1))
                            st = vspool.tile([128, 512], f16, tag="vst")
                            nc.vector.tensor_copy(st[:], ps[:])
                            nc.gpsimd.dma_start(
                                v_sp[ts(tt, 128), ds(vch * 512, 512)], st[:])
                # A-QK: qk^T[j, t] = Wqk^T[c, j].T @ x^T[c, t]  (+ bias on q)
                # Group order q0, k0, q1, k1 so attention heads 0-3 unblock
                # after two groups.
                with tc.tile_pool(name="wqk", bufs=18) as wpool, \
                     tc.tile_pool(name="qkst", bufs=6) as spool, \
                     tc.tile_pool(name="psA", bufs=8, space="PSUM") as pspool:
                    for jg_i, jg in enumerate((0, 2, 1, 3)):
                        wts = []
                        for ci in range(NCT):
                            w_ = wpool.tile([128, 512], bf16, tag="w",
                                            name=f"w{jg}_{ci}")
                            nc.sync.dma_start(
                                w_[:], WqkT[ts(ci, 128), ds(jg * 512, 512)])
                            wts.append(w_)
                        for jj in range(4):
                            jt = jg * 4 + jj
                            pss_l = [pspool.tile([128, 512], f32, tag="ps",
                                                 name=f"ps{jt}_{t2}")
                                     for t2 in range(NTC)]
                            for ci in range(NCT):
                                for tch in range(NTC):
                                    nc.tensor.matmul(
                                        pss_l[tch][:], wts[ci][:, ts(jj, 128)],
                                        xt[ci][:, ts(tch, 512)],
                                        start=(ci == 0), stop=(ci == NCT - 1))
                            for tch in range(NTC):
                                st = spool.tile([128, 512], bf16, tag="st")
                                if jt < JQ // 128:   # q tile: bias add
                                    nc.scalar.activation(
                                        st[:], pss_l[tch][:], Ident,
                                        bias=bq_t[jt][:, 0:1])
                                else:                # k tile: plain copy
                                    nc.scalar.copy(st[:], pss_l[tch][:])
                                nc.gpsimd.dma_start(
                                    qk_sp[ts(jt, 128), ts(tch, 512)], st[:])
                        if jg_i == 1:
                            # prefetch head-0 attention inputs mid-phase-A
                            nc.sync.dma_start(
                                kt0[:], qk_sp[ds(JQ, 128), :])
                            nc.sync.dma_start(
                                vh0[:].rearrange("p (n d) -> p n d", d=128),
                                v_sp[:, ds(0, 128)].rearrange(
                                    "(n p) d -> p n d", p=128))
                            nc.sync.dma_start(qc0[:], qk_sp[ds(0, 128),
                                                            ds(0, 512)])

            # ---------------- Phase B: attention ----------------
            with tc.tile_pool(name="ysb", bufs=1) as ypool, \
                 tc.tile_pool(name="woc", bufs=1) as wopool:
                y_t = [ypool.tile([128, T], bf16, tag=f"y{h}", name=f"y{h}")
                       for h in range(HG)]
                # out-projection weights stream in during phase B so phase C
                # starts without a DMA stall
                wo_t = []
                for ch in range(HG):
                    w_ = wopool.tile([128, C], bf16, tag=f"wo{ch}",
                                     name=f"wo{ch}")
                    nc.sync.dma_start(w_[:], WoT[ts(ch, 128), :])
                    wo_t.append(w_)
                with tc.tile_pool(name="kv", bufs=4) as kvpool, \
                     tc.tile_pool(name="qc", bufs=3) as qcpool, \
                     tc.tile_pool(name="es", bufs=10) as espool, \
                     tc.tile_pool(name="acc", bufs=2) as accpool, \
                     tc.tile_pool(name="nrm", bufs=6) as npool, \
                     tc.tile_pool(name="psS", bufs=5, space="PSUM") as pss, \
                     tc.tile_pool(name="psY", bufs=2, space="PSUM") as psy, \
                     tc.tile_pool(name="psD", bufs=1, space="PSUM") as psd:

                    # Slot-based software pipeline: one slot per score tile.
                    # The AV matmul for a tile runs LAG slots after its
                    # scores matmul so the exp (ACT) + mask (DVE) chain never
                    # stalls the PE. The softmax denominator is accumulated
                    # on the DVE (f16 2x) and reduced with one PE matmul per
                    # chunk; the normalization chain (dens -> reciprocal ->
                    # broadcast -> multiply) is spread over the next chunk's
                    # slots.
                    LAG = 3
                    avq = []        # (due_slot, es, ps_y, vh_sl, fi, la)
                    events = {}     # slot -> [thunk]

                    def at_slot(s, fn):
                        events.setdefault(s, []).append(fn)

                    def mk_dens(acc_, st):
                        # ones MATRIX stationary: out[m,q] = sum_k acc[k,q]
                        # for every m -- the denominator arrives already
                        # broadcast across partitions, same matmul cost.
                        def fn():
                            ps_d = psd.tile([128, 512], f32, tag="pd",
                                            name="pd")
                            nc.tensor.matmul(ps_d[:], ones_sq16, acc_[:],
                                             start=True, stop=True)
                            st["ps_d"] = ps_d
                        return fn

                    def mk_rec(st):
                        def fn():
                            rb = npool.tile([128, 512], f32, tag="rb",
                                            name="rb")
                            nc.vector.reciprocal_approx_fast(
                                rb[:], st["ps_d"][:])
                            st["rb"] = rb
                        return fn

                    def mk_mult(ps_y_, h_, ci_, st):
                        def fn():
                            nc.vector.tensor_tensor(
                                y_t[h_][:, ds(ci_ * 512, 512)], ps_y_[:],
                                st["rb"][:], mult)
                        return fn

                    def pump(slot):
                        for fn in events.pop(slot, []):
                            fn()
                        while avq and avq[0][0] <= slot:
                            _, e_, py_, vs_, fi_, la_ = avq.pop(0)
                            nc.tensor.matmul(py_[:], vs_, e_[:],
                                             start=fi_, stop=la_)

                    # prefetched attention inputs, one head / one chunk ahead
                    kts = {0: kt0}
                    vhs = {0: vh0}
                    qcs = {(0, 0): qc0}

                    def fetch_head(h):
                        if h >= HG or h in kts:
                            return
                        kt_ = kvpool.tile([128, T], bf16, tag="kt")
                        nc.sync.dma_start(
                            kt_[:], qk_sp[ds(JQ + h * 128, 128), :])
                        vh_ = kvpool.tile([128, T], f16, tag="vh")
                        nc.sync.dma_start(
                            vh_[:].rearrange("p (n d) -> p n d", d=128),
                            v_sp[:, ds(h * 128, 128)].rearrange(
                                "(n p) d -> p n d", p=128))
                        kts[h] = kt_
                        vhs[h] = vh_

                    def fetch_qc(h, ci):
                        if h >= HG or (h, ci) in qcs:
                            return
                        qc_ = qcpool.tile([128, 512], bf16, tag="qc")
                        nc.sync.dma_start(
                            qc_[:], qk_sp[ds(h * 128, 128), ds(ci * 512, 512)])
                        qcs[(h, ci)] = qc_

                    slot = 0
                    for h in range(HG):
                        fetch_head(h)
                        kt, vh = kts[h], vhs[h]
                        fetch_head(h + 1)
                        for ci in range(NTC):
                            fetch_qc(h, ci)
                            qc = qcs.pop((h, ci))
                            if ci + 1 < NTC:
                                fetch_qc(h, ci + 1)
                            else:
                                fetch_qc(h + 1, 0)
                            ps_y = psy.tile([128, 512], f32, tag="py")
                            acc = accpool.tile([128, 512], f16, tag="acc",
                                               name="acc")
                            njt = 4 * (ci + 1)
                            for jt in range(njt):
                                ps_s = pss.tile([128, 512], f32, tag="pss")
                                nc.tensor.matmul(ps_s[:], kt[:, ts(jt, 128)],
                                                 qc[:], start=True, stop=True)
                                pump(slot)
                                es = espool.tile([128, 512], f16, tag="es")
                                off = jt * 128 - ci * 512
                                if off > 0:
                                    # diagonal band: columns < off are fully
                                    # masked -- zero them and exp only the
                                    # valid range (saves ACT, the B-phase
                                    # bottleneck)
                                    nc.vector.memset(es[:, 0:off], 0.0)
                                    nc.scalar.activation(es[:, ds(off, 512 - off)],
                                                         ps_s[:, ds(off, 512 - off)],
                                                         Exp)
                                else:
                                    nc.scalar.activation(es[:], ps_s[:], Exp)
                                if off >= 0:
                                    # triangular 128-wide block at the
                                    # diagonal is the only mixed region
                                    oi = off // 128
                                    nc.vector.tensor_tensor(
                                        es[:, ds(off, 128)],
                                        es[:, ds(off, 128)],
                                        mask_t[:, ds(oi * 512 + off, 128)],
                                        mult)
                                if jt == 0:
                                    nc.vector.tensor_copy(acc[:], es[:])
                                elif off > 0:
                                    nc.vector.tensor_tensor(
                                        acc[:, ds(off, 512 - off)],
                                        acc[:, ds(off, 512 - off)],
                                        es[:, ds(off, 512 - off)], add)
                                else:
                                    nc.vector.tensor_tensor(acc[:], acc[:],
                                                            es[:], add)
                                avq.append((slot + LAG, es, ps_y,
                                            vh[:, ts(jt, 128)],
                                            jt == 0, jt == njt - 1))
                                slot += 1
                            st = {}
                            at_slot(slot + 1, mk_dens(acc, st))
                            at_slot(slot + 3, mk_rec(st))
                            at_slot(slot + 5, mk_mult(ps_y, h, ci, st))
                    # drain the pipeline
                    while avq or events:
                        pump(slot)
                        slot += 1

                # ---------------- Phase C: out-projection ----------------
                with tc.tile_pool(name="ost", bufs=6) as ospool, \
                     tc.tile_pool(name="psO", bufs=8, space="PSUM") as pso:
                    for tt in range(NTT):
                        po_l = [pso.tile([128, 512], f32, tag="po",
                                         name=f"po{tt}_{c2}")
                                for c2 in range(C // 512)]
                        for ch in range(HG):
                            for cch in range(C // 512):
                                nc.tensor.matmul(
                                    po_l[cch][:], y_t[ch][:, ts(tt, 128)],
                                    wo_t[ch][:, ds(cch * 512, 512)],
                                    start=(ch == 0), stop=(ch == HG - 1))
                        for cch in range(C // 512):
                            st = ospool.tile([128, 512], bf16, tag="ost")
                            nc.vector.tensor_copy(st[:], po_l[cch][:])
                            nc.gpsimd.dma_start(
                                o[ts(tt, 128), ds(cch * 512, 512)], st[:])
            kv0cm.__exit__(None, None, None)
    nc.compile()
    return nc


def _rope_matrix():
    inv_freq = 1.0 / (ROPE_BASE ** (np.arange(0, D, 2, dtype=np.float64) / D))
    freqs = np.float64(T) * inv_freq
    emb = np.concatenate([freqs, freqs])
    cos, sin = np.cos(emb), np.sin(emb)
    R = np.zeros((D, D))
    for j in range(D):
        R[j, j] = cos[j]
    for j in range(64):
        R[j, 2 * j + 1] += -sin[j]
    for j in range(64, 128):
        R[j, 2 * (j - 64)] += sin[j]
    return R


def _host_mask():
    m = np.zeros((128, 4 * 512), dtype=np.float16)
    ii = np.arange(512)
    jj = np.arange(128)
    for oi, off in enumerate((0, 128, 256, 384)):
        m[:, oi * 512:(oi + 1) * 512] = (jj[:, None] + off <= ii[None, :])
    return m


def kernel(x, Wqkv, bqkv, Wout, bout):
    import ml_dtypes
    from concourse.bass_utils import run_bass_kernel_spmd

    bfloat16 = ml_dtypes.bfloat16

    if "nc" not in _CACHE:
        _CACHE["nc"] = _build_nc()
    nc = _CACHE["nc"]

    x = np.asarray(x, dtype=np.float32)
    Wqkv64 = np.asarray(Wqkv, dtype=np.float64)
    bqkv64 = np.asarray(bqkv, dtype=np.float64)
    Wout64 = np.asarray(Wout, dtype=np.float64)
    bout64 = np.asarray(bout, dtype=np.float64)

    R = _rope_matrix()
    scale = 1.0 / np.sqrt(np.float64(D))
    Wq = Wqkv64[:C].reshape(H, D, C)
    Wk = Wqkv64[C:2 * C].reshape(H, D, C)
    Wv = Wqkv64[2 * C:].reshape(H, D, C)
    bqv = bqkv64[:C].reshape(H, D)
    bv = bqkv64[2 * C:]

    Wq_f = np.einsum('jk,hkc->hjc', R, Wq) * scale
    bq_f = np.einsum('jk,hk->hj', R, bqv) * scale
    Wk_f = np.einsum('jk,hkc->hjc', R, Wk)
    bias_final = (bout64 + Wout64 @ bv).astype(np.float32)

    mask = _host_mask()
    in_maps = []
    xTb = [np.ascontiguousarray(x[b].T).astype(bfloat16) for b in range(B)]
    shard = {}
    for hg in range(2):
        hs = slice(hg * HG, (hg + 1) * HG)
        wqk = np.concatenate(
            [Wq_f[hs].reshape(JQ, C), Wk_f[hs].reshape(JQ, C)], axis=0)
        shard[hg] = dict(
            WqkT=np.ascontiguousarray(wqk.T).astype(bfloat16),
            WvT=np.ascontiguousarray(
                Wv[hs].reshape(JQ, C).T).astype(bfloat16),
            WoT=np.ascontiguousarray(
                Wout64[:, hg * JQ:(hg + 1) * JQ].T).astype(bfloat16),
            bq=bq_f[hs].reshape(JQ, 1).astype(np.float32),
        )
    for core in range(NCORES):
        b, hg = core // 2, core % 2
        in_maps.append(dict(xT=xTb[b], msk=mask,
                            ones_in=np.ones((128, 128), dtype=np.float32),
                            ones16_in=np.ones((128, 2), dtype=np.float16),
                            **shard[hg]))

    res = run_bass_kernel_spmd(nc, in_maps, core_ids=list(range(NCORES)),
                               **_CACHE.get("run_kwargs", {}))
    _CACHE["last_result"] = res
    out = np.empty((B, T, C), dtype=np.float32)
    for b in range(B):
        out[b] = (res.results[2 * b]["o"].astype(np.float32)
                  + res.results[2 * b + 1]["o"].astype(np.float32)
                  + bias_final)
    return out


# revision 32
# speedup vs baseline: 1.0496x; 1.0066x over previous
"""Trainium2 Bass kernel for a causal attention head block (B=4, T=2048, C=2048,
H=16, D=128) with RoPE (single fixed position, folded into weights on host).

Sharding: 8 cores = 4 batches x 2 head-groups (8 heads each).
Per core: QKV projection, causal attention with exp-softmax (no max
subtraction -- scores are small), out-projection partial. Host sums the two
per-batch partials and adds the folded bias.

Data types: matmul operands in bf16/f16 (same PE speed as f32r, half the
DMA/SBUF), accumulation in f32 PSUM. Softmax denominator is accumulated on
the DVE in f16 (2x mode) so the PE runs one ones-matmul per 512-query chunk
instead of one per exp tile.
"""
import numpy as np

B, T, C, H, D = 4, 2048, 2048, 16, 128
ROPE_BASE = 10000.0
HG = H // 2            # heads per core: 8
JQ = HG * D            # 1024 q (or k, or v) channels per core
NCORES = 8
NCT = C // 128         # 16 contraction tiles
NTT = T // 128         # 16 token tiles
NTC = T // 512         # 4 token chunks of 512

_CACHE = {}


def _build_nc():
    import concourse.bass as bass
    import concourse.mybir as mybir
    import concourse.tile as tile
    from concourse import bacc

    f32, f32r = mybir.dt.float32, mybir.dt.float32r
    f16, bf16 = mybir.dt.float16, mybir.dt.bfloat16
    ds, ts = bass.ds, bass.ts
    Exp = mybir.ActivationFunctionType.Exp
    Ident = mybir.ActivationFunctionType.Identity
    mult = mybir.AluOpType.mult
    add = mybir.AluOpType.add

    nc = bacc.Bacc("TRN2", target_bir_lowering=False, debug=False)
    xT = nc.dram_tensor("xT", [C, T], bf16, kind="ExternalInput").ap()
    WqkT = nc.dram_tensor("WqkT", [C, 2 * JQ], bf16, kind="ExternalInput").ap()
    WvT = nc.dram_tensor("WvT", [C, JQ], bf16, kind="ExternalInput").ap()
    WoT = nc.dram_tensor("WoT", [JQ, C], bf16, kind="ExternalInput").ap()
    bq = nc.dram_tensor("bq", [JQ, 1], f32, kind="ExternalInput").ap()
    msk = nc.dram_tensor("msk", [128, 4 * 512], f16, kind="ExternalInput").ap()
    ones16_in = nc.dram_tensor("ones16_in", [128, 2], f16, kind="ExternalInput").ap()
    ones_in = nc.dram_tensor("ones_in", [128, 128], f32r, kind="ExternalInput").ap()
    qk_sp = nc.dram_tensor("qk_sp", [2 * JQ, T], bf16).ap()  # [q;k]^T spill
    v_sp = nc.dram_tensor("v_sp", [T, JQ], f16).ap()         # V spill [t, jv]
    o = nc.dram_tensor("o", [T, C], bf16, kind="ExternalOutput").ap()

    with tile.TileContext(nc) as tc:
        with tc.tile_pool(name="const", bufs=1) as cpool:
            mask_t = cpool.tile([128, 4 * 512], f16, tag="mask")
            nc.sync.dma_start(mask_t[:], msk[:])
            ones2d = cpool.tile([128, 128], f32r, tag="ones2d")
            nc.sync.dma_start(ones2d[:], ones_in[:])
            ones16 = cpool.tile([128, 2], f16, tag="ones16")
            nc.sync.dma_start(ones16[:], ones16_in[:])
            ones_sq16 = mask_t[:, 384:512]
            ones_row = ones2d[0:1, :]
            bq_t = []
            for j in range(JQ // 128):
                t_ = cpool.tile([128, 1], f32, tag=f"bq{j}")
                nc.sync.dma_start(t_[:], bq[ts(j, 128), :])
                bq_t.append(t_)

            # ---------------- Phase A: projections ----------------
            # Head-0 attention inputs live below the xt stack slot so their
            # DMAs can prefetch during phase A (disjoint SBUF space).
            kv0cm = tc.tile_pool(name="kv0", bufs=1)
            kv0pool = kv0cm.__enter__()
            kt0 = kv0pool.tile([128, T], bf16, tag="kt0", name="kt0")
            vh0 = kv0pool.tile([128, T], f16, tag="vh0", name="vh0")
            qc0 = kv0pool.tile([128, 512], bf16, tag="qc0", name="qc0")
            with tc.tile_pool(name="xt", bufs=1) as xpool:
                # A-V first: V[t, jv] = x^T[c, t].T @ Wv^T[c, jv].
                # Interleave xT and first-chunk WvT DMAs so PE starts early.
                xt = [None] * NCT
                with tc.tile_pool(name="wv", bufs=19) as wvpool, \
                     tc.tile_pool(name="vst", bufs=4) as vspool, \
                     tc.tile_pool(name="psV", bufs=8, space="PSUM") as psvpool:
                    for vch in range(JQ // 512):     # 2 chunks of 512
                        wvs = []
                        for ci in range(NCT):
                            if vch == 0:
                                t_ = xpool.tile([128, T], bf16, tag=f"x{ci}",
                                                name=f"x{ci}")
                                # first token-column block right away so the
                                # tt=0 accumulation chain unblocks after
                                # ~3 MB of DMA (x[:,0:128] + wv only)
                                nc.sync.dma_start(t_[:, 0:128],
                                                 xT[ts(ci, 128), 0:128])
                                xt[ci] = t_
                            w_ = wvpool.tile([128, 512], bf16, tag="wv",
                                             name=f"wv{vch}_{ci}")
                            nc.sync.dma_start(
                                w_[:], WvT[ts(ci, 128), ds(vch * 512, 512)])
                            wvs.append(w_)
                        if vch == 0:
                            for ci in range(NCT):
                                nc.sync.dma_start(xt[ci][:, 128:512],
                                                  xT[ts(ci, 128), 128:512])
                            for tcol in range(1, NTC):
                                for ci in range(NCT):
                                    nc.sync.dma_start(
                                        xt[ci][:, ts(tcol, 512)],
                                        xT[ts(ci, 128), ts(tcol, 512)])
                        for tt in range(NTT):
                            ps = psvpool.tile([128, 512], f32, tag="psv")
                            for ci in range(NCT):
                                nc.tensor.matmul(
                                    ps[:], xt[ci][:, ts(tt, 128)], wvs[ci][:],
                                    start=(ci == 0), stop=(ci == NCT - 1))
                            st = vspool.tile([128, 512], f16, tag="vst")
                            nc.vector.tensor_copy(st[:], ps[:])
                            nc.gpsimd.dma_start(
                                v_sp[ts(tt, 128), ds(vch * 512, 512)], st[:])
                # A-QK: qk^T[j, t] = Wqk^T[c, j].T @ x^T[c, t]  (+ bias on q)
                # Group order q0, k0, q1, k1 so attention heads 0-3 unblock
                # after two groups.
                with tc.tile_pool(name="wqk", bufs=18) as wpool, \
                     tc.tile_pool(name="qkst", bufs=6) as spool, \
                     tc.tile_pool(name="psA", bufs=8, space="PSUM") as pspool:
                    for jg_i, jg in enumerate((0, 2, 1, 3)):
                        wts = []
                        for ci in range(NCT):
                            w_ = wpool.tile([128, 512], bf16, tag="w",
                                            name=f"w{jg}_{ci}")
                            nc.sync.dma_start(
                                w_[:], WqkT[ts(ci, 128), ds(jg * 512, 512)])
                            wts.append(w_)
                        for jj in range(4):
                            jt = jg * 4 + jj
                            pss_l = [pspool.tile([128, 512], f32, tag="ps",
                                                 name=f"ps{jt}_{t2}")
                                     for t2 in range(NTC)]
                            for ci in range(NCT):
                                for tch in range(NTC):
                                    nc.tensor.matmul(
                                        pss_l[tch][:], wts[ci][:, ts(jj, 128)],
                                        xt[ci][:, ts(tch, 512)],
                                        start=(ci == 0), stop=(ci == NCT - 1))
                            for tch in range(NTC):
                                st = spool.tile([128, 512], bf16, tag="st")
                                if jt < JQ // 128:   # q tile: bias add
                                    nc.scalar.activation(
                                        st[:], pss_l[tch][:], Ident,
                                        bias=bq_t[jt][:, 0:1])
                                else:                # k tile: plain copy
                                    nc.scalar.copy(st[:], pss_l[tch][:])
                                nc.gpsimd.dma_start(
                                    qk_sp[ts(jt, 128), ts(tch, 512)], st[:])
                        if jg_i == 1:
                            # prefetch head-0 attention inputs mid-phase-A
                            nc.sync.dma_start(
                                kt0[:], qk_sp[ds(JQ, 128), :])
                            nc.sync.dma_start(
                                vh0[:].rearrange("p (n d) -> p n d", d=128),
                                v_sp[:, ds(0, 128)].rearrange(
                                    "(n p) d -> p n d", p=128))
                            nc.sync.dma_start(qc0[:], qk_sp[ds(0, 128),
                                                            ds(0, 512)])

            # ---------------- Phase B: attention ----------------
            with tc.tile_pool(name="ysb", bufs=1) as ypool, \
                 tc.tile_pool(name="woc", bufs=1) as wopool:
                y_t = [ypool.tile([128, T], bf16, tag=f"y{h}", name=f"y{h}")
                       for h in range(HG)]
                # out-projection weights stream in during phase B so phase C
                # starts without a DMA stall
                wo_t = []
                for ch in range(HG):
                    w_ = wopool.tile([128, C], bf16, tag=f"wo{ch}",
                                     name=f"wo{ch}")
                    nc.sync.dma_start(w_[:], WoT[ts(ch, 128), :])
                    wo_t.append(w_)
                with tc.tile_pool(name="kv", bufs=4) as kvpool, \
                     tc.tile_pool(name="qc", bufs=3) as qcpool, \
                     tc.tile_pool(name="es", bufs=10) as espool, \
                     tc.tile_pool(name="acc", bufs=2) as accpool, \
                     tc.tile_pool(name="nrm", bufs=6) as npool, \
                     tc.tile_pool(name="psS", bufs=4, space="PSUM") as pss, \
                     tc.tile_pool(name="psY", bufs=2, space="PSUM") as psy, \
                     tc.tile_pool(name="psD", bufs=2, space="PSUM") as psd:

                    # Slot-based software pipeline: one slot per score tile.
                    # The AV matmul for a tile runs LAG slots after its
                    # scores matmul so the exp (ACT) + mask (DVE) chain never
                    # stalls the PE. The softmax denominator is accumulated
                    # on the DVE (f16 2x) and reduced with one PE matmul per
                    # chunk; the normalization chain (dens -> reciprocal ->
                    # broadcast -> multiply) is spread over the next chunk's
                    # slots.
                    LAG = 3
                    avq = []        # (due_slot, es, ps_y, vh_sl, fi, la)
                    events = {}     # slot -> [thunk]

                    def at_slot(s, fn):
                        events.setdefault(s, []).append(fn)

                    def mk_dens(acc_, st):
                        # ones MATRIX stationary: out[m,q] = sum_k acc[k,q]
                        # for every m -- the denominator arrives already
                        # broadcast across partitions, same matmul cost.
                        def fn():
                            ps_d = psd.tile([128, 512], f32, tag="pd",
                                            name="pd")
                            nc.tensor.matmul(ps_d[:], ones_sq16, acc_[:],
                                             start=True, stop=True)
                            st["ps_d"] = ps_d
                        return fn

                    def mk_rec(st):
                        def fn():
                            rb = npool.tile([128, 512], f32, tag="rb",
                                            name="rb")
                            nc.vector.reciprocal_approx_fast(
                                rb[:], st["ps_d"][:])
                            st["rb"] = rb
                        return fn

                    def mk_mult(ps_y_, h_, ci_, st):
                        def fn():
                            nc.vector.tensor_tensor(
                                y_t[h_][:, ds(ci_ * 512, 512)], ps_y_[:],
                                st["rb"][:], mult)
                        return fn

                    def pump(slot):
                        for fn in events.pop(slot, []):
                            fn()
                        while avq and avq[0][0] <= slot:
                            _, e_, py_, vs_, fi_, la_ = avq.pop(0)
                            nc.tensor.matmul(py_[:], vs_, e_[:],
                                             start=fi_, stop=la_)

                    # prefetched attention inputs, one head / one chunk ahead
                    kts = {0: kt0}
                    vhs = {0: vh0}
                    qcs = {(0, 0): qc0}

                    def fetch_head(h):
                        if h >= HG or h in kts:
                            return
                        kt_ = kvpool.tile([128, T], bf16, tag="kt")
                        nc.sync.dma_start(
                            kt_[:], qk_sp[ds(JQ + h * 128, 128), :])
                        vh_ = kvpool.tile([128, T], f16, tag="vh")
                        nc.sync.dma_start(
                            vh_[:].rearrange("p (n d) -> p n d", d=128),
                            v_sp[:, ds(h * 128, 128)].rearrange(
                                "(n p) d -> p n d", p=128))
                        kts[h] = kt_
                        vhs[h] = vh_

                    def fetch_qc(h, ci):
                        if h >= HG or (h, ci) in qcs:
                            return
                        qc_ = qcpool.tile([128, 512], bf16, tag="qc")
                        nc.sync.dma_start(
                            qc_[:], qk_sp[ds(h * 128, 128), ds(ci * 512, 512)])
                        qcs[(h, ci)] = qc_

                    slot = 0
                    for h in range(HG):
                        fetch_head(h)
                        kt, vh = kts[h], vhs[h]
                        fetch_head(h + 1)
                        for ci in range(NTC):
                            fetch_qc(h, ci)
                            qc = qcs.pop((h, ci))
                            if ci + 1 < NTC:
                                fetch_qc(h, ci + 1)
                            else:
                                fetch_qc(h + 1, 0)
                            ps_y = psy.tile([128, 512], f32, tag="py")
                            acc = accpool.tile([128, 512], f16, tag="acc",
                                               name="acc")
                            njt = 4 * (ci + 1)
                            for jt in range(njt):
                                ps_s = pss.tile([128, 512], f32, tag="pss")
                                nc.tensor.matmul(ps_s[:], kt[:, ts(jt, 128)],
                                                 qc[:], start=True, stop=True)
                                pump(slot)
                                es = espool.tile([128, 512], f16, tag="es")
                                off = jt * 128 - ci * 512
                                if off > 0:
                                    # diagonal band: columns < off are fully
                                    # masked -- zero them and exp only the
                                    # valid range (saves ACT, the B-phase
                                    # bottleneck)
                                    nc.vector.memset(es[:, 0:off], 0.0)
                                    nc.scalar.activation(es[:, ds(off, 512 - off)],
                                                         ps_s[:, ds(off, 512 - off)],
                                                         Exp)
                                else:
                                    nc.scalar.activation(es[:], ps_s[:], Exp)
                                if off >= 0:
                                    # triangular 128-wide block at the
                                    # diagonal is the only mixed region
                                    oi = off // 128
                                    nc.vector.tensor_tensor(
                                        es[:, ds(off, 128)],
                                        es[:, ds(off, 128)],
                                        mask_t[:, ds(oi * 512 + off, 128)],
                                        mult)
                                if jt == 0:
                                    nc.vector.tensor_copy(acc[:], es[:])
                                elif off > 0:
                                    nc.vector.tensor_tensor(
                                        acc[:, ds(off, 512 - off)],
                                        acc[:, ds(off, 512 - off)],
                                        es[:, ds(off, 512 - off)], add)
                                else:
                                    nc.vector.tensor_tensor(acc[:], acc[:],
                                                            es[:], add)
                                avq.append((slot + LAG, es, ps_y,
                                            vh[:, ts(jt, 128)],
                                            jt == 0, jt == njt - 1))
                                slot += 1
                            st = {}
                            at_slot(slot + 1, mk_dens(acc, st))
                            at_slot(slot + 3, mk_rec(st))
                            at_slot(slot + 5, mk_mult(ps_y, h, ci, st))
                    # drain the pipeline
                    while avq or events:
                        pump(slot)
                        slot += 1

                # ---------------- Phase C: out-projection ----------------
                with tc.tile_pool(name="ost", bufs=6) as ospool, \
                     tc.tile_pool(name="psO", bufs=8, space="PSUM") as pso:
                    for tt in range(NTT):
                        po_l = [pso.tile([128, 512], f32, tag="po",
                                         name=f"po{tt}_{c2}")
                                for c2 in range(C // 512)]
                        for ch in range(HG):
                            for cch in range(C // 512):
                                nc.tensor.matmul(
                                    po_l[cch][:], y_t[ch][:, ts(tt, 128)],
                                    wo_t[ch][:, ds(cch * 512, 512)],
                                    start=(ch == 0), stop=(ch == HG - 1))
                        for cch in range(C // 512):
                            st = ospool.tile([128, 512], bf16, tag="ost")
                            nc.vector.tensor_copy(st[:], po_l[cch][:])
                            nc.sync.dma_start(
                                o[ts(tt, 128), ds(cch * 512, 512)], st[:])
            kv0cm.__exit__(None, None, None)
    nc.compile()
    return nc


def _rope_matrix():
    inv_freq = 1.0 / (ROPE_BASE ** (np.arange(0, D, 2, dtype=np.float64) / D))
    freqs = np.float64(T) * inv_freq
    emb = np.concatenate([freqs, freqs])
    cos, sin = np.cos(emb), np.sin(emb)
    R = np.zeros((D, D))
    for j in range(D):
        R[j, j] = cos[j]
    for j in range(64):
        R[j, 2 * j + 1] += -sin[j]
    for j in range(64, 128):
        R[j, 2 * (j - 64)] += sin[j]
    return R


def _host_mask():
    m = np.zeros((128, 4 * 512), dtype=np.float16)
    ii = np.arange(512)
    jj = np.arange(128)
    for oi, off in enumerate((0, 128, 256, 384)):
        m[:, oi * 512:(oi + 1) * 512] = (jj[:, None] + off <= ii[None, :])
    return m


def kernel(x, Wqkv, bqkv, Wout, bout):
    import ml_dtypes
    from concourse.bass_utils import run_bass_kernel_spmd

    bfloat16 = ml_dtypes.bfloat16

    if "nc" not in _CACHE:
        _CACHE["nc"] = _build_nc()
    nc = _CACHE["nc"]

    x = np.asarray(x, dtype=np.float32)
    Wqkv64 = np.asarray(Wqkv, dtype=np.float64)
    bqkv64 = np.asarray(bqkv, dtype=np.float64)
    Wout64 = np.asarray(Wout, dtype=np.float64)
    bout64 = np.asarray(bout, dtype=np.float64)

    R = _rope_matrix()
    scale = 1.0 / np.sqrt(np.float64(D))
    Wq = Wqkv64[:C].reshape(H, D, C)
    Wk = Wqkv64[C:2 * C].reshape(H, D, C)
    Wv = Wqkv64[2 * C:].reshape(H, D, C)
    bqv = bqkv64[:C].reshape(H, D)
    bv = bqkv64[2 * C:]

    Wq_f = np.einsum('jk,hkc->hjc', R, Wq) * scale
    bq_f = np.einsum('jk,hk->hj', R, bqv) * scale
    Wk_f = np.einsum('jk,hkc->hjc', R, Wk)
    bias_final = (bout64 + Wout64 @ bv).astype(np.float32)

    mask = _host_mask()
    in_maps = []
    xTb = [np.ascontiguousarray(x[b].T).astype(bfloat16) for b in range(B)]
    shard = {}
    for hg in range(2):
        hs = slice(hg * HG, (hg + 1) * HG)
        wqk = np.concatenate(
            [Wq_f[hs].reshape(JQ, C), Wk_f[hs].reshape(JQ, C)], axis=0)
        shard[hg] = dict(
            WqkT=np.ascontiguousarray(wqk.T).astype(bfloat16),
            WvT=np.ascontiguousarray(
                Wv[hs].reshape(JQ, C).T).astype(bfloat16),
            WoT=np.ascontiguousarray(
                Wout64[:, hg * JQ:(hg + 1) * JQ].T).astype(bfloat16),
            bq=bq_f[hs].reshape(JQ, 1).astype(np.float32),
        )
    for core in range(NCORES):
        b, hg = core // 2, core % 2
        in_maps.append(dict(xT=xTb[b], msk=mask,
                            ones_in=np.ones((128, 128), dtype=np.float32),
                            ones16_in=np.ones((128, 2), dtype=np.float16),
                            **shard[hg]))

    res = run_bass_kernel_spmd(nc, in_maps, core_ids=list(range(NCORES)),
                               **_CACHE.get("run_kwargs", {}))
    _CACHE["last_result"] = res
    out = np.empty((B, T, C), dtype=np.float32)
    for b in range(B):
        out[b] = (res.results[2 * b]["o"].astype(np.float32)
                  + res.results[2 * b + 1]["o"].astype(np.float32)
                  + bias_final)
    return out


# revision 33
# speedup vs baseline: 1.0590x; 1.0090x over previous
"""Trainium2 Bass kernel for a causal attention head block (B=4, T=2048, C=2048,
H=16, D=128) with RoPE (single fixed position, folded into weights on host).

Sharding: 8 cores = 4 batches x 2 head-groups (8 heads each).
Per core: QKV projection, causal attention with exp-softmax (no max
subtraction -- scores are small), out-projection partial. Host sums the two
per-batch partials and adds the folded bias.

Data types: matmul operands in bf16/f16 (same PE speed as f32r, half the
DMA/SBUF), accumulation in f32 PSUM. Softmax denominator is accumulated on
the DVE in f16 (2x mode) so the PE runs one ones-matmul per 512-query chunk
instead of one per exp tile.
"""
import numpy as np

B, T, C, H, D = 4, 2048, 2048, 16, 128
ROPE_BASE = 10000.0
HG = H // 2            # heads per core: 8
JQ = HG * D            # 1024 q (or k, or v) channels per core
NCORES = 8
NCT = C // 128         # 16 contraction tiles
NTT = T // 128         # 16 token tiles
NTC = T // 512         # 4 token chunks of 512

_CACHE = {}


def _build_nc():
    import concourse.bass as bass
    import concourse.mybir as mybir
    import concourse.tile as tile
    from concourse import bacc

    f32, f32r = mybir.dt.float32, mybir.dt.float32r
    f16, bf16 = mybir.dt.float16, mybir.dt.bfloat16
    ds, ts = bass.ds, bass.ts
    Exp = mybir.ActivationFunctionType.Exp
    Ident = mybir.ActivationFunctionType.Identity
    mult = mybir.AluOpType.mult
    add = mybir.AluOpType.add

    nc = bacc.Bacc("TRN2", target_bir_lowering=False, debug=False)
    xT = nc.dram_tensor("xT", [C, T], bf16, kind="ExternalInput").ap()
    WqkT = nc.dram_tensor("WqkT", [C, 2 * JQ], bf16, kind="ExternalInput").ap()
    WvT = nc.dram_tensor("WvT", [C, JQ], bf16, kind="ExternalInput").ap()
    WoT = nc.dram_tensor("WoT", [JQ, C], bf16, kind="ExternalInput").ap()
    bq = nc.dram_tensor("bq", [JQ, 1], f32, kind="ExternalInput").ap()
    msk = nc.dram_tensor("msk", [128, 4 * 512], f16, kind="ExternalInput").ap()
    ones16_in = nc.dram_tensor("ones16_in", [128, 2], f16, kind="ExternalInput").ap()
    ones_in = nc.dram_tensor("ones_in", [128, 128], f32r, kind="ExternalInput").ap()
    qk_sp = nc.dram_tensor("qk_sp", [2 * JQ, T], bf16).ap()  # [q;k]^T spill
    v_sp = nc.dram_tensor("v_sp", [T, JQ], f16).ap()         # V spill [t, jv]
    o = nc.dram_tensor("o", [T, C], bf16, kind="ExternalOutput").ap()

    with tile.TileContext(nc) as tc:
        with tc.tile_pool(name="const", bufs=1) as cpool:
            mask_t = cpool.tile([128, 4 * 512], f16, tag="mask")
            nc.sync.dma_start(mask_t[:], msk[:])
            ones2d = cpool.tile([128, 128], f32r, tag="ones2d")
            nc.sync.dma_start(ones2d[:], ones_in[:])
            ones16 = cpool.tile([128, 2], f16, tag="ones16")
            nc.sync.dma_start(ones16[:], ones16_in[:])
            ones_sq16 = mask_t[:, 384:512]
            ones_row = ones2d[0:1, :]
            bq_t = []
            for j in range(JQ // 128):
                t_ = cpool.tile([128, 1], f32, tag=f"bq{j}")
                nc.sync.dma_start(t_[:], bq[ts(j, 128), :])
                bq_t.append(t_)

            # ---------------- Phase A: projections ----------------
            # Head-0 attention inputs live below the xt stack slot so their
            # DMAs can prefetch during phase A (disjoint SBUF space).
            kv0cm = tc.tile_pool(name="kv0", bufs=1)
            kv0pool = kv0cm.__enter__()
            kt0 = kv0pool.tile([128, T], bf16, tag="kt0", name="kt0")
            vh0 = kv0pool.tile([128, T], f16, tag="vh0", name="vh0")
            qc0 = kv0pool.tile([128, 512], bf16, tag="qc0", name="qc0")
            with tc.tile_pool(name="xt", bufs=1) as xpool, \
                 tc.tile_pool(name="wqk", bufs=18) as wpool:
                # A-V first: V[t, jv] = x^T[c, t].T @ Wv^T[c, jv].
                # Interleave xT and first-chunk WvT DMAs so PE starts early.
                xt = [None] * NCT
                wts0 = []
                with tc.tile_pool(name="wv", bufs=19) as wvpool, \
                     tc.tile_pool(name="vst", bufs=4) as vspool, \
                     tc.tile_pool(name="psV", bufs=8, space="PSUM") as psvpool:
                    for vch in range(JQ // 512):     # 2 chunks of 512
                        wvs = []
                        for ci in range(NCT):
                            if vch == 0:
                                t_ = xpool.tile([128, T], bf16, tag=f"x{ci}",
                                                name=f"x{ci}")
                                # first token-column block right away so the
                                # tt=0 accumulation chain unblocks after
                                # ~3 MB of DMA (x[:,0:128] + wv only)
                                nc.sync.dma_start(t_[:, 0:128],
                                                 xT[ts(ci, 128), 0:128])
                                xt[ci] = t_
                            w_ = wvpool.tile([128, 512], bf16, tag="wv",
                                             name=f"wv{vch}_{ci}")
                            nc.sync.dma_start(
                                w_[:], WvT[ts(ci, 128), ds(vch * 512, 512)])
                            wvs.append(w_)
                        if vch == 0:
                            for ci in range(NCT):
                                nc.sync.dma_start(xt[ci][:, 128:512],
                                                  xT[ts(ci, 128), 128:512])
                            for tcol in range(1, NTC):
                                for ci in range(NCT):
                                    nc.sync.dma_start(
                                        xt[ci][:, ts(tcol, 512)],
                                        xT[ts(ci, 128), ts(tcol, 512)])
                        if vch == 1:
                            # prefetch the first A-QK weight group during
                            # A-V's compute tail
                            for ci in range(NCT):
                                w_ = wpool.tile([128, 512], bf16, tag="w",
                                                name=f"w0_{ci}")
                                nc.sync.dma_start(
                                    w_[:], WqkT[ts(ci, 128), ds(0, 512)])
                                wts0.append(w_)
                        for tt in range(NTT):
                            ps = psvpool.tile([128, 512], f32, tag="psv")
                            for ci in range(NCT):
                                nc.tensor.matmul(
                                    ps[:], xt[ci][:, ts(tt, 128)], wvs[ci][:],
                                    start=(ci == 0), stop=(ci == NCT - 1))
                            st = vspool.tile([128, 512], f16, tag="vst")
                            nc.vector.tensor_copy(st[:], ps[:])
                            nc.gpsimd.dma_start(
                                v_sp[ts(tt, 128), ds(vch * 512, 512)], st[:])
                # A-QK: qk^T[j, t] = Wqk^T[c, j].T @ x^T[c, t]  (+ bias on q)
                # Group order q0, k0, q1, k1 so attention heads 0-3 unblock
                # after two groups.
                with tc.tile_pool(name="qkst", bufs=6) as spool, \
                     tc.tile_pool(name="psA", bufs=8, space="PSUM") as pspool:
                    for jg_i, jg in enumerate((0, 2, 1, 3)):
                        if jg_i == 0:
                            wts = wts0
                        else:
                            wts = []
                            for ci in range(NCT):
                                w_ = wpool.tile([128, 512], bf16, tag="w",
                                                name=f"w{jg}_{ci}")
                                nc.sync.dma_start(
                                    w_[:], WqkT[ts(ci, 128), ds(jg * 512, 512)])
                                wts.append(w_)
                        for jj in range(4):
                            jt = jg * 4 + jj
                            pss_l = [pspool.tile([128, 512], f32, tag="ps",
                                                 name=f"ps{jt}_{t2}")
                                     for t2 in range(NTC)]
                            for ci in range(NCT):
                                for tch in range(NTC):
                                    nc.tensor.matmul(
                                        pss_l[tch][:], wts[ci][:, ts(jj, 128)],
                                        xt[ci][:, ts(tch, 512)],
                                        start=(ci == 0), stop=(ci == NCT - 1))
                            for tch in range(NTC):
                                st = spool.tile([128, 512], bf16, tag="st")
                                if jt < JQ // 128:   # q tile: bias add
                                    nc.scalar.activation(
                                        st[:], pss_l[tch][:], Ident,
                                        bias=bq_t[jt][:, 0:1])
                                else:                # k tile: plain copy
                                    nc.scalar.copy(st[:], pss_l[tch][:])
                                nc.gpsimd.dma_start(
                                    qk_sp[ts(jt, 128), ts(tch, 512)], st[:])
                        if jg_i == 1:
                            # prefetch head-0 attention inputs mid-phase-A
                            nc.sync.dma_start(
                                kt0[:], qk_sp[ds(JQ, 128), :])
                            nc.sync.dma_start(
                                vh0[:].rearrange("p (n d) -> p n d", d=128),
                                v_sp[:, ds(0, 128)].rearrange(
                                    "(n p) d -> p n d", p=128))
                            nc.sync.dma_start(qc0[:], qk_sp[ds(0, 128),
                                                            ds(0, 512)])

            # ---------------- Phase B: attention ----------------
            with tc.tile_pool(name="ysb", bufs=1) as ypool, \
                 tc.tile_pool(name="woc", bufs=1) as wopool:
                y_t = [ypool.tile([128, T], bf16, tag=f"y{h}", name=f"y{h}")
                       for h in range(HG)]
                # out-projection weights stream in during phase B so phase C
                # starts without a DMA stall
                wo_t = []
                for ch in range(HG):
                    w_ = wopool.tile([128, C], bf16, tag=f"wo{ch}",
                                     name=f"wo{ch}")
                    nc.sync.dma_start(w_[:], WoT[ts(ch, 128), :])
                    wo_t.append(w_)
                with tc.tile_pool(name="kv", bufs=4) as kvpool, \
                     tc.tile_pool(name="qc", bufs=3) as qcpool, \
                     tc.tile_pool(name="es", bufs=10) as espool, \
                     tc.tile_pool(name="acc", bufs=2) as accpool, \
                     tc.tile_pool(name="nrm", bufs=6) as npool, \
                     tc.tile_pool(name="psS", bufs=4, space="PSUM") as pss, \
                     tc.tile_pool(name="psY", bufs=2, space="PSUM") as psy, \
                     tc.tile_pool(name="psD", bufs=2, space="PSUM") as psd:

                    # Slot-based software pipeline: one slot per score tile.
                    # The AV matmul for a tile runs LAG slots after its
                    # scores matmul so the exp (ACT) + mask (DVE) chain never
                    # stalls the PE. The softmax denominator is accumulated
                    # on the DVE (f16 2x) and reduced with one PE matmul per
                    # chunk; the normalization chain (dens -> reciprocal ->
                    # broadcast -> multiply) is spread over the next chunk's
                    # slots.
                    LAG = 3
                    avq = []        # (due_slot, es, ps_y, vh_sl, fi, la)
                    events = {}     # slot -> [thunk]

                    def at_slot(s, fn):
                        events.setdefault(s, []).append(fn)

                    def mk_dens(acc_, st):
                        # ones MATRIX stationary: out[m,q] = sum_k acc[k,q]
                        # for every m -- the denominator arrives already
                        # broadcast across partitions, same matmul cost.
                        def fn():
                            ps_d = psd.tile([128, 512], f32, tag="pd",
                                            name="pd")
                            nc.tensor.matmul(ps_d[:], ones_sq16, acc_[:],
                                             start=True, stop=True)
                            st["ps_d"] = ps_d
                        return fn

                    def mk_rec(st):
                        def fn():
                            rb = npool.tile([128, 512], f32, tag="rb",
                                            name="rb")
                            nc.vector.reciprocal_approx_fast(
                                rb[:], st["ps_d"][:])
                            st["rb"] = rb
                        return fn

                    def mk_mult(ps_y_, h_, ci_, st):
                        def fn():
                            nc.vector.tensor_tensor(
                                y_t[h_][:, ds(ci_ * 512, 512)], ps_y_[:],
                                st["rb"][:], mult)
                        return fn

                    def pump(slot):
                        for fn in events.pop(slot, []):
                            fn()
                        while avq and avq[0][0] <= slot:
                            _, e_, py_, vs_, fi_, la_ = avq.pop(0)
                            nc.tensor.matmul(py_[:], vs_, e_[:],
                                             start=fi_, stop=la_)

                    # prefetched attention inputs, one head / one chunk ahead
                    kts = {0: kt0}
                    vhs = {0: vh0}
                    qcs = {(0, 0): qc0}

                    def fetch_head(h):
                        if h >= HG or h in kts:
                            return
                        kt_ = kvpool.tile([128, T], bf16, tag="kt")
                        nc.sync.dma_start(
                            kt_[:], qk_sp[ds(JQ + h * 128, 128), :])
                        vh_ = kvpool.tile([128, T], f16, tag="vh")
                        nc.sync.dma_start(
                            vh_[:].rearrange("p (n d) -> p n d", d=128),
                            v_sp[:, ds(h * 128, 128)].rearrange(
                                "(n p) d -> p n d", p=128))
                        kts[h] = kt_
                        vhs[h] = vh_

                    def fetch_qc(h, ci):
                        if h >= HG or (h, ci) in qcs:
                            return
                        qc_ = qcpool.tile([128, 512], bf16, tag="qc")
                        nc.sync.dma_start(
                            qc_[:], qk_sp[ds(h * 128, 128), ds(ci * 512, 512)])
                        qcs[(h, ci)] = qc_

                    slot = 0
                    for h in range(HG):
                        fetch_head(h)
                        kt, vh = kts[h], vhs[h]
                        fetch_head(h + 1)
                        for ci in range(NTC):
                            fetch_qc(h, ci)
                            qc = qcs.pop((h, ci))
                            if ci + 1 < NTC:
                                fetch_qc(h, ci + 1)
                            else:
                                fetch_qc(h + 1, 0)
                            ps_y = psy.tile([128, 512], f32, tag="py")
                            acc = accpool.tile([128, 512], f16, tag="acc",
                                               name="acc")
                            njt = 4 * (ci + 1)
                            for jt in range(njt):
                                ps_s = pss.tile([128, 512], f32, tag="pss")
                                nc.tensor.matmul(ps_s[:], kt[:, ts(jt, 128)],
                                                 qc[:], start=True, stop=True)
                                pump(slot)
                                es = espool.tile([128, 512], f16, tag="es")
                                off = jt * 128 - ci * 512
                                if off > 0:
                                    # diagonal band: columns < off are fully
                                    # masked -- zero them and exp only the
                                    # valid range (saves ACT, the B-phase
                                    # bottleneck)
                                    nc.vector.memset(es[:, 0:off], 0.0)
                                    nc.scalar.activation(es[:, ds(off, 512 - off)],
                                                         ps_s[:, ds(off, 512 - off)],
                                                         Exp)
                                else:
                                    nc.scalar.activation(es[:], ps_s[:], Exp)
                                if off >= 0:
                                    # triangular 128-wide block at the
                                    # diagonal is the only mixed region
                                    oi = off // 128
                                    nc.vector.tensor_tensor(
                                        es[:, ds(off, 128)],
                                        es[:, ds(off, 128)],
                                        mask_t[:, ds(oi * 512 + off, 128)],
                                        mult)
                                if jt == 0:
                                    nc.vector.tensor_copy(acc[:], es[:])
                                elif off > 0:
                                    nc.vector.tensor_tensor(
                                        acc[:, ds(off, 512 - off)],
                                        acc[:, ds(off, 512 - off)],
                                        es[:, ds(off, 512 - off)], add)
                                else:
                                    nc.vector.tensor_tensor(acc[:], acc[:],
                                                            es[:], add)
                                avq.append((slot + LAG, es, ps_y,
                                            vh[:, ts(jt, 128)],
                                            jt == 0, jt == njt - 1))
                                slot += 1
                            st = {}
                            at_slot(slot + 1, mk_dens(acc, st))
                            at_slot(slot + 3, mk_rec(st))
                            at_slot(slot + 5, mk_mult(ps_y, h, ci, st))
                    # drain the pipeline
                    while avq or events:
                        pump(slot)
                        slot += 1

                # ---------------- Phase C: out-projection ----------------
                with tc.tile_pool(name="ost", bufs=6) as ospool, \
                     tc.tile_pool(name="psO", bufs=8, space="PSUM") as pso:
                    for tt in range(NTT):
                        po_l = [pso.tile([128, 512], f32, tag="po",
                                         name=f"po{tt}_{c2}")
                                for c2 in range(C // 512)]
                        for ch in range(HG):
                            for cch in range(C // 512):
                                nc.tensor.matmul(
                                    po_l[cch][:], y_t[ch][:, ts(tt, 128)],
                                    wo_t[ch][:, ds(cch * 512, 512)],
                                    start=(ch == 0), stop=(ch == HG - 1))
                        for cch in range(C // 512):
                            st = ospool.tile([128, 512], bf16, tag="ost")
                            nc.vector.tensor_copy(st[:], po_l[cch][:])
                            nc.sync.dma_start(
                                o[ts(tt, 128), ds(cch * 512, 512)], st[:])
            kv0cm.__exit__(None, None, None)
    nc.compile()
    return nc


def _rope_matrix():
    inv_freq = 1.0 / (ROPE_BASE ** (np.arange(0, D, 2, dtype=np.float64) / D))
    freqs = np.float64(T) * inv_freq
    emb = np.concatenate([freqs, freqs])
    cos, sin = np.cos(emb), np.sin(emb)
    R = np.zeros((D, D))
    for j in range(D):
        R[j, j] = cos[j]
    for j in range(64):
        R[j, 2 * j + 1] += -sin[j]
    for j in range(64, 128):
        R[j, 2 * (j - 64)] += sin[j]
    return R


def _host_mask():
    m = np.zeros((128, 4 * 512), dtype=np.float16)
    ii = np.arange(512)
    jj = np.arange(128)
    for oi, off in enumerate((0, 128, 256, 384)):
        m[:, oi * 512:(oi + 1) * 512] = (jj[:, None] + off <= ii[None, :])
    return m


def kernel(x, Wqkv, bqkv, Wout, bout):
    import ml_dtypes
    from concourse.bass_utils import run_bass_kernel_spmd

    bfloat16 = ml_dtypes.bfloat16

    if "nc" not in _CACHE:
        _CACHE["nc"] = _build_nc()
    nc = _CACHE["nc"]

    x = np.asarray(x, dtype=np.float32)
    Wqkv64 = np.asarray(Wqkv, dtype=np.float64)
    bqkv64 = np.asarray(bqkv, dtype=np.float64)
    Wout64 = np.asarray(Wout, dtype=np.float64)
    bout64 = np.asarray(bout, dtype=np.float64)

    R = _rope_matrix()
    scale = 1.0 / np.sqrt(np.float64(D))
    Wq = Wqkv64[:C].reshape(H, D, C)
    Wk = Wqkv64[C:2 * C].reshape(H, D, C)
    Wv = Wqkv64[2 * C:].reshape(H, D, C)
    bqv = bqkv64[:C].reshape(H, D)
    bv = bqkv64[2 * C:]

    Wq_f = np.einsum('jk,hkc->hjc', R, Wq) * scale
    bq_f = np.einsum('jk,hk->hj', R, bqv) * scale
    Wk_f = np.einsum('jk,hkc->hjc', R, Wk)
    bias_final = (bout64 + Wout64 @ bv).astype(np.float32)

    mask = _host_mask()
    in_maps = []
    xTb = [np.ascontiguousarray(x[b].T).astype(bfloat16) for b in range(B)]
    shard = {}
    for hg in range(2):
        hs = slice(hg * HG, (hg + 1) * HG)
        wqk = np.concatenate(
            [Wq_f[hs].reshape(JQ, C), Wk_f[hs].reshape(JQ, C)], axis=0)
        shard[hg] = dict(
            WqkT=np.ascontiguousarray(wqk.T).astype(bfloat16),
            WvT=np.ascontiguousarray(
                Wv[hs].reshape(JQ, C).T).astype(bfloat16),
            WoT=np.ascontiguousarray(
                Wout64[:, hg * JQ:(hg + 1) * JQ].T).astype(bfloat16),
            bq=bq_f[hs].reshape(JQ, 1).astype(np.float32),
        )
    for core in range(NCORES):
        b, hg = core // 2, core % 2
        in_maps.append(dict(xT=xTb[b], msk=mask,
                            ones_in=np.ones((128, 128), dtype=np.float32),
                            ones16_in=np.ones((128, 2), dtype=np.float16),
                            **shard[hg]))

    res = run_bass_kernel_spmd(nc, in_maps, core_ids=list(range(NCORES)),
                               **_CACHE.get("run_kwargs", {}))
    _CACHE["last_result"] = res
    out = np.empty((B, T, C), dtype=np.float32)
    for b in range(B):
        out[b] = (res.results[2 * b]["o"].astype(np.float32)
                  + res.results[2 * b + 1]["o"].astype(np.float32)
                  + bias_final)
    return out
